# revision 9
# baseline (speedup 1.0000x reference)
"""Causal self-attention (B=4, T=1024, C=2048, H=16, rotary) on 8 trn2 cores.

Sharding: core c = 2*b + g handles batch b, head-group g (heads 8g..8g+7).
 - QKV projection in transposed layout (Q^T/K^T = [d, T]; V natural [T, d]).
 - RoPE fully on DVE: rotate-half via 64-partition tensor ops with
   mismatched in/out partition windows (no PE permutation matmul).
 - Scores transposed S^T = K^T.Q -> [k, q]; softmax without max-subtraction;
   causal masking via multiplicative 0/1 bf16 masks on diagonal blocks.
 - Softmax denominator: pairwise DVE tree then ones[128,128] matmul
   accumulates column sums broadcast across partitions; 1/d via DVE
   reciprocal (frees the scalar engine of the Ln/Exp chain).
 - c_proj by ReduceScatter: each core computes partial = y_local @
   w_proj[local 1024 rows, all 2048 cols] in waves as local heads finish,
   stashes f32 partials, bounces bf16 partials to DRAM, and a chunked
   ReduceScatter(add) over core pairs writes the final (bf16) output
   shard directly. No matmul ever waits on a collective.
 - DMA issue is split across the two HWDGE queues (sync + scalar) so the
   QKV ramp is not serialized on one queue.
 - Emission interleaves attention blocks into the QKV / c_proj matmul
   streams at sub-microsecond granularity so the ACT-gated softmax never
   stalls the in-order PE queue.
All matmuls bf16 (fp32 PSUM accumulation).
"""

import math

import numpy as np
import ml_dtypes

BF16 = ml_dtypes.bfloat16

B, T, C = 4, 1024, 2048
H = 16  # total heads
D = C // H  # 128 head dim
HG = 8  # heads per group (per core)
N_CORES = 8
ROPE_BASE = 10000.0

TUNE = {
    "ps_a": 2,
    "ps_b": 2,
    "ps_y": 2,
    "ps_s": 2,
    "p_sb_bufs": 5,
    "attn_per_qkv": (2, 2, 3),  # attn yields consumed per qkv microstep
    "rs_chunks": 4,
}

_PROGRAM_CACHE = {}


def _build_program(num_devices=N_CORES):
    import concourse.mybir as mybir
    import concourse.tile as tile
    from concourse import bacc
    from concourse.bass import ts

    f32 = mybir.dt.float32
    bf16 = mybir.dt.bfloat16
    AF = mybir.ActivationFunctionType

    nc = bacc.Bacc(trn_type="TRN2", num_devices=num_devices, debug=False)

    # ---- per-core I/O ----
    xT = nc.dram_tensor("xT", [C, T], bf16, kind="ExternalInput")  # x[b].T
    wqkv = nc.dram_tensor("wqkv", [C, 3 * HG * D], bf16, kind="ExternalInput")
    bqk = nc.dram_tensor("bqk", [128, 16], f32, kind="ExternalInput")
    bv = nc.dram_tensor("bv", [1, HG * D], f32, kind="ExternalInput")
    # full-height rope tables: cos2 = [cos; cos], sinB = [sin; -sin]
    cosT = nc.dram_tensor("cosT", [D, T], bf16, kind="ExternalInput")
    sinT = nc.dram_tensor("sinT", [D, T], bf16, kind="ExternalInput")
    maskT = nc.dram_tensor("maskT", [128, 4, 512], bf16, kind="ExternalInput")
    # w_proj rows for the local head-group, all 2048 output columns
    wproj = nc.dram_tensor("wproj", [C // 2, C], bf16, kind="ExternalInput")
    # full bias on rank 0 of each pair, zeros on rank 1
    bproj = nc.dram_tensor("bproj", [1, C], f32, kind="ExternalInput")
    out = nc.dram_tensor("out", [T, C // 2], bf16, kind="ExternalOutput")

    xT_r = xT.ap().rearrange("(ct p) t -> p ct t", p=128)  # [128, 16, 1024]
    wqkv_r = wqkv.ap().rearrange("(ct p) j -> p ct j", p=128)  # [128, 16, 3072]
    wproj_r = wproj.ap().rearrange("(jt p) c -> p jt c", p=128)  # [128, 8, 2048]

    scale = 1.0 / math.sqrt(D)
    NRS = TUNE["rs_chunks"]

    with tile.TileContext(nc) as tc:
        with (
            tc.tile_pool(name="const", bufs=1) as const,
            tc.tile_pool(name="persist", bufs=1) as persist,
            tc.tile_pool(name="wp_pool", bufs=1) as wp_pool,
            tc.tile_pool(name="ps_a", bufs=TUNE["ps_a"], space="PSUM") as psA,
            tc.tile_pool(name="ps_b", bufs=TUNE["ps_b"], space="PSUM") as psB,
            tc.tile_pool(name="ps_y", bufs=TUNE["ps_y"], space="PSUM") as psY,
            tc.tile_pool(name="ps_sum", bufs=TUNE["ps_s"], space="PSUM") as psS,
            tc.tile_pool(name="work", bufs=4) as work,
            tc.tile_pool(name="dram", bufs=1, space="DRAM") as drampool,
        ):
            # ---- persistent activations ----
            qf = persist.tile([128, HG, T], bf16)  # [d, h, t] rotated Q^T
            kf = persist.tile([128, HG, T], bf16)  # [d, h, t] rotated K^T
            v_all = persist.tile([128, 8, HG * D], bf16)  # [t_in, tt, j]
            yT = persist.tile([128, HG, T], bf16)  # [d, h, t] normalized att out

            # DRAM staging for the ReduceScatter: chunk-major so each chunk's
            # [2, T//NRS, C//2] block is contiguous (collectives require a
            # contiguous input pattern); plane p holds the partial
            # contribution to rank-p's output columns.
            rs_in = drampool.tile([NRS, 2, T // NRS, C // 2], bf16, name="rs_in")
            rs_out = drampool.tile([T, C // 2], bf16, name="rs_out")

            with (
                tc.tile_pool(name="xpool", bufs=1) as xpool,
                tc.tile_pool(name="wpool", bufs=2) as wpool,
                tc.tile_pool(name="rope", bufs=1) as rope_pool,
            ):
                xs = xpool.tile([128, 16, T], bf16, name="xs")
                wts = {}

                def load_chunk(chunk, eng, granules=1):
                    wt = wpool.tile([128, 16, 512], bf16, tag="wt", name="wt")
                    wts[chunk] = wt
                    cslice = slice(chunk * 512, (chunk + 1) * 512)
                    if granules == 1:
                        eng.dma_start(out=wt, in_=wqkv_r[:, :, cslice])
                    else:
                        step = 16 // granules
                        for g in range(granules):
                            cts = slice(g * step, (g + 1) * step)
                            eng.dma_start(
                                out=wt[:, cts, :], in_=wqkv_r[:, cts, cslice]
                            )

                # critical-path loads first: xs + chunk0 interleaved on sync,
                # biases/rope tables then chunk2/chunk4 on scalar
                wt0 = wpool.tile([128, 16, 512], bf16, tag="wt", name="wt")
                wts[0] = wt0
                for q in range(8):
                    cts = slice(2 * q, 2 * q + 2)
                    nc.sync.dma_start(out=xs[:, cts, :], in_=xT_r[:, cts, :])
                    nc.sync.dma_start(out=wt0[:, cts, :], in_=wqkv_r[:, cts, 0:512])
                bqk_sb = const.tile([128, 16], f32)
                nc.scalar.dma_start(out=bqk_sb, in_=bqk.ap())
                cos_sb = rope_pool.tile([128, T], bf16)
                nc.scalar.dma_start(out=cos_sb, in_=cosT.ap())
                sin_sb = rope_pool.tile([128, T], bf16)
                nc.scalar.dma_start(out=sin_sb, in_=sinT.ap())
                ones128 = const.tile([128, 128], bf16)
                nc.vector.memset(ones128, 1.0)
                load_chunk(2, nc.scalar, granules=8)
                load_chunk(4, nc.scalar, granules=2)
                mask_sb = const.tile([128, 4, 512], bf16)
                nc.scalar.dma_start(out=mask_sb, in_=maskT.ap())
                bv_bc = const.tile([128, HG * D], f32)
                nc.scalar.dma_start(
                    out=bv_bc, in_=bv.ap().to_broadcast([128, HG * D])
                )
                bp_bc = const.tile([128, C], f32)
                nc.scalar.dma_start(
                    out=bp_bc, in_=bproj.ap().to_broadcast([128, C])
                )

                wp = wp_pool.tile([128, 8, C], bf16, name="wp")

                # ---------- emission generators ----------
                def qk_steps(chunk):
                    """Q or K projection + rope, one (jj, th) microstep per
                    yield (16 matmuls)."""
                    wt = wts[chunk]
                    for jj in range(4):
                        jt = chunk * 4 + jj  # q: 0-7, k: 8-15
                        h = jt % 8
                        dest_all = qf if jt < 8 else kf
                        for th in range(2):
                            ps = psA.tile([128, 512], f32, tag="ps", name="ps")
                            for ct in range(16):
                                nc.tensor.matmul(
                                    ps,
                                    lhsT=wt[:, ct, jj * 128 : (jj + 1) * 128],
                                    rhs=xs[:, ct, ts(th, 512)],
                                    start=(ct == 0),
                                    stop=(ct == 15),
                                )
                            raw = work.tile(
                                [128, 512], bf16, tag="raw", name="raw", bufs=3
                            )
                            nc.scalar.activation(
                                raw, ps, AF.Identity,
                                bias=bqk_sb[:, jt : jt + 1],
                            )
                            dest = dest_all[:, h, ts(th, 512)]
                            # rotate-half on DVE: t2[0:64] = raw[64:]*(-sin),
                            # t2[64:] = raw[:64]*sin; dest = raw*cos + t2
                            t2 = work.tile(
                                [128, 512], bf16, tag="t2", name="t2", bufs=3
                            )
                            ss = ts(th, 512)
                            nc.vector.tensor_mul(
                                t2[0:64, :], raw[64:128, :], sin_sb[64:128, ss]
                            )
                            nc.vector.tensor_mul(
                                t2[64:128, :], raw[0:64, :], sin_sb[0:64, ss]
                            )
                            nc.vector.tensor_mul(dest, raw, cos_sb[:, ss])
                            nc.vector.tensor_add(dest, dest, t2)
                            yield

                def v_steps(chunk):
                    jc = chunk - 4  # 0 or 1
                    wt = wts[chunk]
                    for tt in range(8):
                        ps = psA.tile([128, 512], f32, tag="ps", name="ps")
                        for ct in range(16):
                            nc.tensor.matmul(
                                ps,
                                lhsT=xs[:, ct, ts(tt, 128)],
                                rhs=wt[:, ct, :],
                                start=(ct == 0),
                                stop=(ct == 15),
                            )
                        nc.vector.tensor_add(
                            v_all[:, tt, jc * 512 : (jc + 1) * 512],
                            ps,
                            bv_bc[:, jc * 512 : (jc + 1) * 512],
                        )
                        yield

                def attn_steps(h):
                    """One yield between a block's score emission and its AV
                    matmul, so interleaved filler work hides the exp latency."""
                    for qc in range(2):
                        n_kt = 4 * (qc + 1)
                        ps_y = psY.tile([128, 512], f32, tag="ps_y", name="ps_y")
                        ps_sum = psS.tile(
                            [128, 512], f32, tag="ps_sum", name="ps_sum"
                        )
                        p_hold = None
                        padd_hold = None
                        for kt in range(n_kt):
                            ps_sc = psB.tile(
                                [128, 512], f32, tag="psb", name="ps_sc"
                            )
                            nc.tensor.matmul(
                                ps_sc,
                                lhsT=kf[:, h, ts(kt, 128)],
                                rhs=qf[:, h, ts(qc, 512)],
                                start=True,
                                stop=True,
                            )
                            p_sb = work.tile(
                                [128, 512], bf16, tag="p_sb", name="p_sb",
                                bufs=TUNE["p_sb_bufs"],
                            )
                            nc.scalar.activation(p_sb, ps_sc, AF.Exp, scale=scale)
                            kt_rel = kt - 4 * qc
                            if 0 <= kt_rel < 4:  # block straddles the diagonal
                                nc.vector.tensor_mul(
                                    p_sb, p_sb, mask_sb[:, kt_rel, :]
                                )
                            if kt % 2 == 0:
                                p_hold = p_sb
                            else:
                                padd = work.tile(
                                    [128, 512], bf16, tag="padd", name="padd",
                                    bufs=3,
                                )
                                nc.vector.tensor_add(padd, p_hold, p_sb)
                                if kt % 4 == 1:
                                    padd_hold = padd
                                else:
                                    pquad = work.tile(
                                        [128, 512], bf16, tag="pquad",
                                        name="pquad", bufs=2,
                                    )
                                    nc.vector.tensor_add(pquad, padd_hold, padd)
                                    # ones[128,128] stationary: column sums
                                    # land broadcast across all partitions
                                    nc.tensor.matmul(
                                        ps_sum,
                                        lhsT=ones128,
                                        rhs=pquad,
                                        start=(kt == 3),
                                        stop=(kt == n_kt - 1),
                                    )
                            yield
                            nc.tensor.matmul(
                                ps_y,
                                lhsT=v_all[:, kt, ts(h, 128)],
                                rhs=p_sb,
                                start=(kt == 0),
                                stop=(kt == n_kt - 1),
                            )
                        rb = work.tile(
                            [128, 512], f32, tag="rb", name="rb", bufs=2
                        )
                        nc.vector.reciprocal(rb, ps_sum)
                        nc.vector.tensor_mul(yT[:, h, ts(qc, 512)], ps_y, rb)
                        yield

                def chain(*gens):
                    for g in gens:
                        yield from g

                def drive(main, filler, per_step):
                    """Advance `filler` per_step[i % len] times after each
                    main step; then drain both."""
                    i = 0
                    for _ in main:
                        for _ in range(per_step[i % len(per_step)]):
                            if next(filler, None) is None:
                                break
                        i += 1
                    for _ in filler:
                        pass

                # ========== phase A: chunks 0, 2, 4 (heads 0-3 + all V lo) ==
                for _ in chain(qk_steps(0), qk_steps(2), v_steps(4)):
                    pass

                # phase B loads on both queues, behind phase-A traffic
                load_chunk(1, nc.sync, granules=8)
                load_chunk(3, nc.scalar, granules=4)
                nc.sync.dma_start(out=wp, in_=wproj_r)
                load_chunk(5, nc.scalar, granules=2)

                # ========== phase B: chunks 1, 3, 5 ⊗ attention h0-3 ========
                drive(
                    chain(qk_steps(1), qk_steps(3), v_steps(5)),
                    chain(*[attn_steps(h) for h in range(4)]),
                    TUNE["attn_per_qkv"],
                )

            # ========== phase C: c_proj waves ⊗ attention h4-7 ==========
            with tc.tile_pool(name="stash_pool", bufs=1) as stash_pool:
                stash = stash_pool.tile([128, 32, 512], f32, name="stash")

                def proj_steps(heads, first, last):
                    """One c_proj tile per yield: chain over `heads`, then
                    stash-accumulate (or final-merge + bounce on `last`)."""
                    for tt in range(8):
                        for cc in range(4):
                            st = tt * 4 + cc
                            ps = psA.tile(
                                [128, 512], f32, tag="ps", name="ps_proj"
                            )
                            for i, h in enumerate(heads):
                                nc.tensor.matmul(
                                    ps,
                                    lhsT=yT[:, h, ts(tt, 128)],
                                    rhs=wp[:, h, ts(cc, 512)],
                                    start=(i == 0),
                                    stop=(i == len(heads) - 1),
                                )
                            if first:
                                nc.vector.tensor_add(
                                    stash[:, st, :], ps, bp_bc[:, ts(cc, 512)]
                                )
                            elif not last:
                                nc.vector.tensor_add(
                                    stash[:, st, :], ps, stash[:, st, :]
                                )
                            else:
                                pb = work.tile(
                                    [128, 512], bf16, tag="pb", name="pb",
                                    bufs=4,
                                )
                                nc.vector.tensor_add(pb, ps, stash[:, st, :])
                                tpc = 8 // NRS  # tt rows per RS chunk
                                nc.sync.dma_start(
                                    out=rs_in[
                                        tt // tpc, cc // 2,
                                        ts(tt % tpc, 128), ts(cc % 2, 512),
                                    ],
                                    in_=pb,
                                )
                            yield
                        if last and tt % (8 // NRS) == (8 // NRS) - 1:
                            k = tt // (8 // NRS)
                            nc.gpsimd.collective_compute(
                                "ReduceScatter",
                                mybir.AluOpType.add,
                                replica_groups=[[0, 1], [2, 3], [4, 5], [6, 7]],
                                ins=[rs_in[k].opt()],
                                outs=[rs_out[ts(k, T // NRS), :].opt()],
                            )
                            # collectives may not write IO tensors directly
                            nc.scalar.dma_start(
                                out=out.ap()[ts(k, T // NRS), :],
                                in_=rs_out[ts(k, T // NRS), :],
                            )

                # W0 {h0-3} ⊗ attn h4,h5,h6 ; W1 {h4-6} ⊗ attn h7 ; W2 {h7}
                drive(
                    proj_steps((0, 1, 2, 3), first=True, last=False),
                    chain(*[attn_steps(h) for h in (4, 5, 6)]),
                    (1, 2, 1),
                )
                drive(
                    proj_steps((4, 5, 6), first=False, last=False),
                    attn_steps(7),
                    (1, 0, 1, 0, 0, 1, 0),
                )
                for _ in proj_steps((7,), first=False, last=True):
                    pass

    # Pin every activation to the one table set holding Exp+Identity
    # (natural_log_exp_and_others) so the set-picker never inserts
    # ACT_TABLE_LOADs mid-kernel.
    import concourse.bacc as bacc_mod

    orig_tables = bacc_mod.get_activation_tables

    def _pinned_tables(arch):
        tabs = orig_tables(arch)
        return {
            name: (funcs if name == "natural_log_exp_and_others" else set())
            for name, funcs in tabs.items()
        }

    bacc_mod.get_activation_tables = _pinned_tables
    try:
        nc.finalize()
    finally:
        bacc_mod.get_activation_tables = orig_tables
    return nc


def _host_inputs(x, w_attn, b_attn, w_proj, b_proj):
    """Build the 8 per-core input maps."""
    x = np.asarray(x, np.float32)
    w_attn = np.asarray(w_attn, np.float32)
    b_attn = np.asarray(b_attn, np.float32)
    w_proj = np.asarray(w_proj, np.float32)
    b_proj = np.asarray(b_proj, np.float32)

    # rope tables, transposed [d, t], full height:
    # dest = raw * cos2 + rot_half(raw) * [-s; s]  with the sign folded into
    # sinB = [s; -s] read at the *source* partition window:
    #   t2[0:64] = raw[64:128] * sinB[64:128] = raw_hi * (-s)
    #   t2[64:128] = raw[0:64] * sinB[0:64]   = raw_lo * s
    inv_freq = 1.0 / (ROPE_BASE ** (np.arange(0, D, 2, dtype=np.float32) / D))
    freqs = np.arange(T, dtype=np.float32)[:, None] * inv_freq[None, :]  # [T, 64]
    c_ = np.ascontiguousarray(np.cos(freqs).T)  # [64, T]
    s_ = np.ascontiguousarray(np.sin(freqs).T)
    cosT = np.concatenate([c_, c_], axis=0).astype(BF16)  # [128, T]
    sinB = np.concatenate([s_, -s_], axis=0).astype(BF16)

    # causal mask blocks, transposed [k, q]: block kt_rel r, q chunk of 512
    k_idx = np.arange(128)
    q_idx = np.arange(512)
    maskT = np.zeros((128, 4, 512), np.float32)
    for r in range(4):
        maskT[:, r, :] = ((r * 128 + k_idx)[:, None] <= q_idx[None, :]).astype(
            np.float32
        )
    maskT = maskT.astype(BF16)

    in_maps = []
    for c in range(N_CORES):
        b, g = divmod(c, 2)
        cs = slice(g * 1024, (g + 1) * 1024)
        wq = w_attn[:, 0:C][:, cs]
        wk = w_attn[:, C : 2 * C][:, cs]
        wv = w_attn[:, 2 * C : 3 * C][:, cs]
        bq = b_attn[0:C][cs]
        bk = b_attn[C : 2 * C][cs]
        bvv = b_attn[2 * C : 3 * C][cs]
        # rank 0 of each pair carries the full c_proj bias, rank 1 zeros
        bp = b_proj if g == 0 else np.zeros_like(b_proj)
        in_maps.append(
            {
                "xT": np.ascontiguousarray(x[b].T).astype(BF16),
                "wqkv": np.concatenate([wq, wk, wv], axis=1).astype(BF16),
                "bqk": np.ascontiguousarray(
                    np.concatenate([bq, bk]).reshape(16, 128).T
                ).astype(np.float32),
                "bv": bvv.reshape(1, 1024).astype(np.float32),
                "cosT": cosT,
                "sinT": sinB,
                "maskT": maskT,
                "wproj": np.ascontiguousarray(w_proj[cs, :]).astype(BF16),
                "bproj": bp.reshape(1, 2048).astype(np.float32),
            }
        )
    return in_maps


def kernel(x, w_attn, b_attn, w_proj, b_proj, _trace=False):
    from concourse.bass_utils import run_bass_kernel_spmd

    if "nc" not in _PROGRAM_CACHE:
        _PROGRAM_CACHE["nc"] = _build_program()
    nc = _PROGRAM_CACHE["nc"]

    in_maps = _host_inputs(x, w_attn, b_attn, w_proj, b_proj)
    res = run_bass_kernel_spmd(
        nc, in_maps, core_ids=list(range(N_CORES)), trace=_trace
    )
    _PROGRAM_CACHE["last_results"] = res

    out = np.zeros((B, T, C), np.float32)
    for c in range(N_CORES):
        b, g = divmod(c, 2)
        out[b, :, g * 1024 : (g + 1) * 1024] = np.asarray(
            res.results[c]["out"], dtype=np.float32
        )
    return out


# revision 15
# speedup vs baseline: 1.0963x; 1.0963x over previous
"""Causal self-attention (B=4, T=1024, C=2048, H=16, rotary) on 8 trn2 cores.

Sharding: core c = 2*b + g handles batch b, head-group g (heads 8g..8g+7).
 - QKV projection in transposed layout (Q^T/K^T = [d, T]; V natural [T, d]).
 - RoPE fully on DVE: rotate-half via 64-partition tensor ops with
   mismatched in/out partition windows (no PE permutation matmul).
 - Scores transposed S^T = K^T.Q -> [k, q]; softmax without max-subtraction;
   causal masking via multiplicative 0/1 bf16 masks on diagonal blocks.
 - Softmax denominator: pairwise adds (padd on DVE, pquad on GpSimd) then
   ones[128,128] matmul accumulates column sums broadcast across
   partitions; 1/d via DVE reciprocal (no scalar-engine Ln/Exp chain).
 - AllGather per head group as heads finish: groups (0,1),(2,3),(4,5),
   (6),(7) so the tail exchange is a single head, then c_proj in waves
   sized to cover each gather: W0{h0-3} interleaved with attention h4-6,
   W1{h4,5} with h7, then W2{h6}, W3{h7} (f32 SBUF stash accumulation,
   final merge -> bf16 -> output DMA per tile).
 - DMA issue is split across the two HWDGE queues (sync + scalar) so the
   QKV ramp is not serialized on one queue.
 - Emission interleaves attention blocks into the QKV / c_proj matmul
   streams at sub-microsecond granularity so the ACT-gated softmax never
   stalls the in-order PE queue.
All matmuls bf16 (fp32 PSUM accumulation).
"""

import math

import numpy as np
import ml_dtypes

BF16 = ml_dtypes.bfloat16

B, T, C = 4, 1024, 2048
H = 16  # total heads
D = C // H  # 128 head dim
HG = 8  # heads per group (per core)
N_CORES = 8
ROPE_BASE = 10000.0

TUNE = {
    "ps_a": 2,
    "ps_b": 2,
    "ps_y": 2,
    "ps_s": 2,
    "p_sb_bufs": 5,
    "attn_per_qkv": (2, 2, 3),  # attn yields consumed per qkv microstep
}

# AllGather groups of local heads; fired when the last head finishes.
AG_HEADS = ((0, 1), (2, 3), (4, 5), (6,), (7,))

_PROGRAM_CACHE = {}


def _build_program(num_devices=N_CORES):
    import concourse.mybir as mybir
    import concourse.tile as tile
    from concourse import bacc
    from concourse.bass import ts

    f32 = mybir.dt.float32
    bf16 = mybir.dt.bfloat16
    AF = mybir.ActivationFunctionType

    nc = bacc.Bacc(trn_type="TRN2", num_devices=num_devices, debug=False)

    # ---- per-core I/O ----
    xT = nc.dram_tensor("xT", [C, T], bf16, kind="ExternalInput")  # x[b].T
    wqkv = nc.dram_tensor("wqkv", [C, 3 * HG * D], bf16, kind="ExternalInput")
    bqk = nc.dram_tensor("bqk", [128, 16], f32, kind="ExternalInput")
    bv = nc.dram_tensor("bv", [1, HG * D], f32, kind="ExternalInput")
    # full-height rope tables: cos2 = [cos; cos], sinB = [sin; -sin]
    cosT = nc.dram_tensor("cosT", [D, T], bf16, kind="ExternalInput")
    sinT = nc.dram_tensor("sinT", [D, T], bf16, kind="ExternalInput")
    maskT = nc.dram_tensor("maskT", [128, 4, 512], bf16, kind="ExternalInput")
    wproj = nc.dram_tensor("wproj", [C, C // 2], bf16, kind="ExternalInput")
    bproj = nc.dram_tensor("bproj", [1, C // 2], f32, kind="ExternalInput")
    out = nc.dram_tensor("out", [T, C // 2], bf16, kind="ExternalOutput")

    xT_r = xT.ap().rearrange("(ct p) t -> p ct t", p=128)  # [128, 16, 1024]
    wqkv_r = wqkv.ap().rearrange("(ct p) j -> p ct j", p=128)  # [128, 16, 3072]
    wproj_r = wproj.ap().rearrange("(jt p) c -> p jt c", p=128)  # [128, 16, 1024]

    scale = 1.0 / math.sqrt(D)

    with tile.TileContext(nc) as tc:
        with (
            tc.tile_pool(name="const", bufs=1) as const,
            tc.tile_pool(name="persist", bufs=1) as persist,
            tc.tile_pool(name="wp_pool", bufs=1) as wp_pool,
            tc.tile_pool(name="ps_a", bufs=TUNE["ps_a"], space="PSUM") as psA,
            tc.tile_pool(name="ps_b", bufs=TUNE["ps_b"], space="PSUM") as psB,
            tc.tile_pool(name="ps_y", bufs=TUNE["ps_y"], space="PSUM") as psY,
            tc.tile_pool(name="ps_sum", bufs=TUNE["ps_s"], space="PSUM") as psS,
            tc.tile_pool(name="work", bufs=4) as work,
            tc.tile_pool(name="dram", bufs=1, space="DRAM") as drampool,
        ):
            # ---- persistent activations ----
            qf = persist.tile([128, HG, T], bf16)  # [d, h, t] rotated Q^T
            kf = persist.tile([128, HG, T], bf16)  # [d, h, t] rotated K^T
            v_all = persist.tile([128, 8, HG * D], bf16)  # [t_in, tt, j]
            yT = persist.tile([128, HG, T], bf16)  # [d, h, t] normalized att out

            # ---- DRAM staging for the collectives ----
            ybounce = drampool.tile([HG * D, T], bf16, name="ybounce")
            ygth_q = [
                drampool.tile([2 * len(hs) * D, T], bf16, name=f"ygth{w}")
                for w, hs in enumerate(AG_HEADS)
            ]
            yb_r = ybounce.rearrange("(h p) t -> p h t", p=128)

            def emit_ag(w):
                hs = AG_HEADS[w]
                rows = slice(D * hs[0], D * (hs[-1] + 1))
                nc.gpsimd.collective_compute(
                    "AllGather",
                    mybir.AluOpType.bypass,
                    replica_groups=[[0, 1], [2, 3], [4, 5], [6, 7]],
                    ins=[ybounce[rows, :].opt()],
                    outs=[ygth_q[w][:].opt()],
                )

            with (
                tc.tile_pool(name="xpool", bufs=1) as xpool,
                tc.tile_pool(name="wpool", bufs=2) as wpool,
                tc.tile_pool(name="rope", bufs=1) as rope_pool,
            ):
                xs = xpool.tile([128, 16, T], bf16, name="xs")
                wts = {}

                def load_chunk(chunk, eng, granules=1):
                    wt = wpool.tile([128, 16, 512], bf16, tag="wt", name="wt")
                    wts[chunk] = wt
                    cslice = slice(chunk * 512, (chunk + 1) * 512)
                    if granules == 1:
                        eng.dma_start(out=wt, in_=wqkv_r[:, :, cslice])
                    else:
                        step = 16 // granules
                        for g in range(granules):
                            cts = slice(g * step, (g + 1) * step)
                            eng.dma_start(
                                out=wt[:, cts, :], in_=wqkv_r[:, cts, cslice]
                            )

                # critical-path loads first: xs + chunk0 interleaved on sync,
                # biases/rope tables then chunk2/chunk4 on scalar
                wt0 = wpool.tile([128, 16, 512], bf16, tag="wt", name="wt")
                wts[0] = wt0
                for q in range(8):
                    cts = slice(2 * q, 2 * q + 2)
                    nc.sync.dma_start(out=xs[:, cts, :], in_=xT_r[:, cts, :])
                    nc.sync.dma_start(out=wt0[:, cts, :], in_=wqkv_r[:, cts, 0:512])
                bqk_sb = const.tile([128, 16], f32)
                nc.scalar.dma_start(out=bqk_sb, in_=bqk.ap())
                cos_sb = rope_pool.tile([128, T], bf16)
                nc.scalar.dma_start(out=cos_sb, in_=cosT.ap())
                sin_sb = rope_pool.tile([128, T], bf16)
                nc.scalar.dma_start(out=sin_sb, in_=sinT.ap())
                ones128 = const.tile([128, 128], bf16)
                nc.vector.memset(ones128, 1.0)
                load_chunk(2, nc.scalar, granules=8)
                load_chunk(4, nc.scalar, granules=2)
                mask_sb = const.tile([128, 4, 512], bf16)
                nc.scalar.dma_start(out=mask_sb, in_=maskT.ap())
                bv_bc = const.tile([128, HG * D], f32)
                nc.scalar.dma_start(
                    out=bv_bc, in_=bv.ap().to_broadcast([128, HG * D])
                )
                bp_bc = const.tile([128, C // 2], f32)
                nc.scalar.dma_start(
                    out=bp_bc, in_=bproj.ap().to_broadcast([128, C // 2])
                )

                wp = wp_pool.tile([128, 16, C // 2], bf16, name="wp")

                # ---------- emission generators ----------
                def qk_steps(chunk):
                    """Q or K projection + rope, one (jj, th) microstep per
                    yield (16 matmuls)."""
                    wt = wts[chunk]
                    for jj in range(4):
                        jt = chunk * 4 + jj  # q: 0-7, k: 8-15
                        h = jt % 8
                        dest_all = qf if jt < 8 else kf
                        for th in range(2):
                            ps = psA.tile([128, 512], f32, tag="ps", name="ps")
                            for ct in range(16):
                                nc.tensor.matmul(
                                    ps,
                                    lhsT=wt[:, ct, jj * 128 : (jj + 1) * 128],
                                    rhs=xs[:, ct, ts(th, 512)],
                                    start=(ct == 0),
                                    stop=(ct == 15),
                                )
                            raw = work.tile(
                                [128, 512], bf16, tag="raw", name="raw", bufs=3
                            )
                            nc.scalar.activation(
                                raw, ps, AF.Identity,
                                bias=bqk_sb[:, jt : jt + 1],
                            )
                            dest = dest_all[:, h, ts(th, 512)]
                            # rotate-half on DVE: t2[0:64] = raw[64:]*(-sin),
                            # t2[64:] = raw[:64]*sin; dest = raw*cos + t2
                            t2 = work.tile(
                                [128, 512], bf16, tag="t2", name="t2", bufs=3
                            )
                            ss = ts(th, 512)
                            nc.vector.tensor_mul(
                                t2[0:64, :], raw[64:128, :], sin_sb[64:128, ss]
                            )
                            nc.vector.tensor_mul(
                                t2[64:128, :], raw[0:64, :], sin_sb[0:64, ss]
                            )
                            nc.vector.tensor_mul(dest, raw, cos_sb[:, ss])
                            nc.vector.tensor_add(dest, dest, t2)
                            yield

                def v_steps(chunk):
                    jc = chunk - 4  # 0 or 1
                    wt = wts[chunk]
                    for tt in range(8):
                        ps = psA.tile([128, 512], f32, tag="ps", name="ps")
                        for ct in range(16):
                            nc.tensor.matmul(
                                ps,
                                lhsT=xs[:, ct, ts(tt, 128)],
                                rhs=wt[:, ct, :],
                                start=(ct == 0),
                                stop=(ct == 15),
                            )
                        nc.vector.tensor_add(
                            v_all[:, tt, jc * 512 : (jc + 1) * 512],
                            ps,
                            bv_bc[:, jc * 512 : (jc + 1) * 512],
                        )
                        yield

                ag_after = {1: 0, 3: 1, 5: 2, 6: 3, 7: 4}
                hooks = {"on_ag": None}  # set to ygs load_group in phase C

                def attn_steps(h):
                    """One yield between a block's score emission and its AV
                    matmul, so interleaved filler work hides the exp latency."""
                    for qc in range(2):
                        n_kt = 4 * (qc + 1)
                        ps_y = psY.tile([128, 512], f32, tag="ps_y", name="ps_y")
                        ps_sum = psS.tile(
                            [128, 512], f32, tag="ps_sum", name="ps_sum"
                        )
                        p_hold = None
                        padd_hold = None
                        for kt in range(n_kt):
                            ps_sc = psB.tile(
                                [128, 512], f32, tag="psb", name="ps_sc"
                            )
                            nc.tensor.matmul(
                                ps_sc,
                                lhsT=kf[:, h, ts(kt, 128)],
                                rhs=qf[:, h, ts(qc, 512)],
                                start=True,
                                stop=True,
                            )
                            p_sb = work.tile(
                                [128, 512], bf16, tag="p_sb", name="p_sb",
                                bufs=TUNE["p_sb_bufs"],
                            )
                            nc.scalar.activation(p_sb, ps_sc, AF.Exp, scale=scale)
                            kt_rel = kt - 4 * qc
                            if 0 <= kt_rel < 4:  # block straddles the diagonal
                                nc.vector.tensor_mul(
                                    p_sb, p_sb, mask_sb[:, kt_rel, :]
                                )
                            if kt % 2 == 0:
                                p_hold = p_sb
                            else:
                                padd = work.tile(
                                    [128, 512], bf16, tag="padd", name="padd",
                                    bufs=3,
                                )
                                nc.vector.tensor_add(padd, p_hold, p_sb)
                                if kt % 4 == 1:
                                    padd_hold = padd
                                else:
                                    pquad = work.tile(
                                        [128, 512], bf16, tag="pquad",
                                        name="pquad", bufs=2,
                                    )
                                    # off the DVE: GpSimd is otherwise idle
                                    nc.gpsimd.tensor_add(pquad, padd_hold, padd)
                                    # ones[128,128] stationary: column sums
                                    # land broadcast across all partitions
                                    nc.tensor.matmul(
                                        ps_sum,
                                        lhsT=ones128,
                                        rhs=pquad,
                                        start=(kt == 3),
                                        stop=(kt == n_kt - 1),
                                    )
                            yield
                            nc.tensor.matmul(
                                ps_y,
                                lhsT=v_all[:, kt, ts(h, 128)],
                                rhs=p_sb,
                                start=(kt == 0),
                                stop=(kt == n_kt - 1),
                            )
                        rb = work.tile(
                            [128, 512], f32, tag="rb", name="rb", bufs=2
                        )
                        nc.vector.reciprocal(rb, ps_sum)
                        nc.vector.tensor_mul(yT[:, h, ts(qc, 512)], ps_y, rb)
                        yield
                    nc.sync.dma_start(out=yb_r[:, h, :], in_=yT[:, h, :])
                    if h in ag_after:
                        emit_ag(ag_after[h])
                        if hooks["on_ag"] is not None:
                            hooks["on_ag"](ag_after[h])

                def chain(*gens):
                    for g in gens:
                        yield from g

                def drive(main, filler, per_step, lead=0):
                    """Advance `filler` per_step[i % len] times after each
                    main step; then drain both."""
                    for _ in range(lead):
                        if next(filler, None) is None:
                            break
                    i = 0
                    for _ in main:
                        for _ in range(per_step[i % len(per_step)]):
                            if next(filler, None) is None:
                                break
                        i += 1
                    for _ in filler:
                        pass

                # ========== phase A: chunks 0, 2, 4 (heads 0-3 + all V lo) ==
                for _ in chain(qk_steps(0), qk_steps(2), v_steps(4)):
                    pass

                # phase B loads on both queues, behind phase-A traffic
                load_chunk(1, nc.sync, granules=8)
                load_chunk(3, nc.scalar, granules=4)
                nc.sync.dma_start(out=wp, in_=wproj_r)
                load_chunk(5, nc.scalar, granules=2)

                # ========== phase B: chunks 1, 3, 5 ⊗ attention h0-3 ========
                drive(
                    chain(qk_steps(1), qk_steps(3), v_steps(5)),
                    chain(*[attn_steps(h) for h in range(4)]),
                    TUNE["attn_per_qkv"],
                )

            # ========== phase C: c_proj waves ⊗ attention h4-7 ==========
            # ygs slot j = global feature block j: slots 8g'+h come from AG
            # group of head h, rank g'.
            with (
                tc.tile_pool(name="stash_pool", bufs=1) as stash_pool,
                tc.tile_pool(name="ygs_pool", bufs=1) as ygs_pool,
            ):
                stash = stash_pool.tile([128, 16, 512], f32, name="stash")
                ygs = ygs_pool.tile([128, 16, T], bf16, name="ygs")

                def load_group(w):
                    hs = AG_HEADS[w]
                    ygth_r = ygth_q[w].rearrange("(j p) t -> p j t", p=128)
                    for r in range(2):
                        for i, h in enumerate(hs):
                            nc.scalar.dma_start(
                                out=ygs[:, 8 * r + h, :],
                                in_=ygth_r[:, r * len(hs) + i, :],
                            )

                load_group(0)
                load_group(1)
                hooks["on_ag"] = load_group

                def proj_steps(heads, first, last):
                    """One c_proj tile per yield: chain over local+remote
                    copies of `heads`, then stash-accumulate (or final-merge
                    + output DMA on `last`)."""
                    jts = [8 * r + h for h in heads for r in range(2)]
                    for tt in range(8):
                        for cc in range(2):
                            st = tt * 2 + cc
                            ps = psA.tile(
                                [128, 512], f32, tag="ps", name="ps_proj"
                            )
                            for i, jt in enumerate(jts):
                                nc.tensor.matmul(
                                    ps,
                                    lhsT=ygs[:, jt, ts(tt, 128)],
                                    rhs=wp[:, jt, ts(cc, 512)],
                                    start=(i == 0),
                                    stop=(i == len(jts) - 1),
                                )
                            if first:
                                nc.vector.tensor_add(
                                    stash[:, st, :], ps, bp_bc[:, ts(cc, 512)]
                                )
                            elif not last:
                                nc.vector.tensor_add(
                                    stash[:, st, :], ps, stash[:, st, :]
                                )
                            else:
                                pb = work.tile(
                                    [128, 512], bf16, tag="pb", name="pb",
                                    bufs=4,
                                )
                                nc.vector.tensor_add(pb, ps, stash[:, st, :])
                                nc.sync.dma_start(
                                    out=out.ap()[ts(tt, 128), ts(cc, 512)],
                                    in_=pb,
                                )
                            yield

                # W0 {h0-3} ⊗ attn h4,h5,h6 ; W1 {h4,5} ⊗ attn h7 ;
                # W2 {h6} ; W3 {h7}  (each wave covers the next AG's latency)
                drive(
                    proj_steps((0, 1, 2, 3), first=True, last=False),
                    chain(*[attn_steps(h) for h in (4, 5, 6)]),
                    (3, 2, 3),
                    lead=4,
                )
                drive(
                    proj_steps((4, 5), first=False, last=False),
                    attn_steps(7),
                    (1, 1, 1, 1, 1, 1, 1, 0),
                )
                for _ in proj_steps((6,), first=False, last=False):
                    pass
                for _ in proj_steps((7,), first=False, last=True):
                    pass

    # Pin every activation to the one table set holding Exp+Identity
    # (natural_log_exp_and_others) so the set-picker never inserts
    # ACT_TABLE_LOADs mid-kernel.
    import concourse.bacc as bacc_mod

    orig_tables = bacc_mod.get_activation_tables

    def _pinned_tables(arch):
        tabs = orig_tables(arch)
        return {
            name: (funcs if name == "natural_log_exp_and_others" else set())
            for name, funcs in tabs.items()
        }

    bacc_mod.get_activation_tables = _pinned_tables
    try:
        nc.finalize()
    finally:
        bacc_mod.get_activation_tables = orig_tables
    return nc


def _host_inputs(x, w_attn, b_attn, w_proj, b_proj):
    """Build the 8 per-core input maps."""
    x = np.asarray(x, np.float32)
    w_attn = np.asarray(w_attn, np.float32)
    b_attn = np.asarray(b_attn, np.float32)
    w_proj = np.asarray(w_proj, np.float32)
    b_proj = np.asarray(b_proj, np.float32)

    # rope tables, transposed [d, t], full height:
    # dest = raw * cos2 + t2 with sinB = [s; -s] read at the source window:
    #   t2[0:64] = raw[64:128] * sinB[64:128] = raw_hi * (-s)
    #   t2[64:128] = raw[0:64] * sinB[0:64]   = raw_lo * s
    inv_freq = 1.0 / (ROPE_BASE ** (np.arange(0, D, 2, dtype=np.float32) / D))
    freqs = np.arange(T, dtype=np.float32)[:, None] * inv_freq[None, :]  # [T, 64]
    c_ = np.ascontiguousarray(np.cos(freqs).T)  # [64, T]
    s_ = np.ascontiguousarray(np.sin(freqs).T)
    cosT = np.concatenate([c_, c_], axis=0).astype(BF16)  # [128, T]
    sinB = np.concatenate([s_, -s_], axis=0).astype(BF16)

    # causal mask blocks, transposed [k, q]: block kt_rel r, q chunk of 512
    k_idx = np.arange(128)
    q_idx = np.arange(512)
    maskT = np.zeros((128, 4, 512), np.float32)
    for r in range(4):
        maskT[:, r, :] = ((r * 128 + k_idx)[:, None] <= q_idx[None, :]).astype(
            np.float32
        )
    maskT = maskT.astype(BF16)

    in_maps = []
    for c in range(N_CORES):
        b, g = divmod(c, 2)
        cs = slice(g * 1024, (g + 1) * 1024)
        wq = w_attn[:, 0:C][:, cs]
        wk = w_attn[:, C : 2 * C][:, cs]
        wv = w_attn[:, 2 * C : 3 * C][:, cs]
        bq = b_attn[0:C][cs]
        bk = b_attn[C : 2 * C][cs]
        bvv = b_attn[2 * C : 3 * C][cs]
        in_maps.append(
            {
                "xT": np.ascontiguousarray(x[b].T).astype(BF16),
                "wqkv": np.concatenate([wq, wk, wv], axis=1).astype(BF16),
                "bqk": np.ascontiguousarray(
                    np.concatenate([bq, bk]).reshape(16, 128).T
                ).astype(np.float32),
                "bv": bvv.reshape(1, 1024).astype(np.float32),
                "cosT": cosT,
                "sinT": sinB,
                "maskT": maskT,
                "wproj": w_proj[:, cs].astype(BF16),
                "bproj": b_proj[cs].reshape(1, 1024).astype(np.float32),
            }
        )
    return in_maps


def kernel(x, w_attn, b_attn, w_proj, b_proj, _trace=False):
    from concourse.bass_utils import run_bass_kernel_spmd

    if "nc" not in _PROGRAM_CACHE:
        _PROGRAM_CACHE["nc"] = _build_program()
    nc = _PROGRAM_CACHE["nc"]

    in_maps = _host_inputs(x, w_attn, b_attn, w_proj, b_proj)
    res = run_bass_kernel_spmd(
        nc, in_maps, core_ids=list(range(N_CORES)), trace=_trace
    )
    _PROGRAM_CACHE["last_results"] = res

    out = np.zeros((B, T, C), np.float32)
    for c in range(N_CORES):
        b, g = divmod(c, 2)
        out[b, :, g * 1024 : (g + 1) * 1024] = np.asarray(
            res.results[c]["out"], dtype=np.float32
        )
    return out


# revision 20
# speedup vs baseline: 1.1230x; 1.0244x over previous
"""Causal self-attention (B=4, T=1024, C=2048, H=16, rotary) on 8 trn2 cores.

Sharding: core c = 2*b + g handles batch b, head-group g (heads 8g..8g+7).
 - QKV projection in transposed layout (Q^T/K^T = [d, T]; V natural [T, d]).
 - RoPE fully on DVE: rotate-half via 64-partition tensor ops with
   mismatched in/out partition windows (no PE permutation matmul).
 - Scores transposed S^T = K^T.Q -> [k, q]; softmax without max-subtraction;
   causal masking via multiplicative 0/1 bf16 masks on diagonal blocks.
 - Softmax denominator: pairwise adds (padd on DVE, pquad on GpSimd) then
   ones[128,128] matmul accumulates column sums broadcast across
   partitions; 1/d via DVE reciprocal (no scalar-engine Ln/Exp chain).
 - AllGather per head group as heads finish: groups (0,1),(2,3),(4,5),
   (6),(7) so the tail exchange is a single head, then c_proj in waves
   sized to cover each gather: W0{h0-3} interleaved with attention h4-6,
   W1{h4,5} with h7, then W2{h6}, W3{h7} (f32 SBUF stash accumulation,
   final merge -> bf16 -> output DMA per tile).
 - DMA issue is split across the two HWDGE queues (sync + scalar) so the
   QKV ramp is not serialized on one queue.
 - Emission interleaves attention blocks into the QKV / c_proj matmul
   streams at sub-microsecond granularity so the ACT-gated softmax never
   stalls the in-order PE queue.
All matmuls bf16 (fp32 PSUM accumulation).
"""

import math

import numpy as np
import ml_dtypes

BF16 = ml_dtypes.bfloat16

B, T, C = 4, 1024, 2048
H = 16  # total heads
D = C // H  # 128 head dim
HG = 8  # heads per group (per core)
N_CORES = 8
ROPE_BASE = 10000.0

TUNE = {
    "ps_a": 2,
    "ps_b": 2,
    "ps_y": 2,
    "ps_s": 2,
    "p_sb_bufs": 5,
    # 1 attn block between 8-matmul qkv half-steps (48 halves, 56 yields)
    "attn_per_qkv": (1, 1, 1, 1, 1, 2),
}

# AllGather groups of local heads; fired when the last head finishes.
AG_HEADS = ((0, 1), (2, 3), (4, 5), (6,), (7,))

_PROGRAM_CACHE = {}


def _build_program(num_devices=N_CORES):
    import concourse.mybir as mybir
    import concourse.tile as tile
    from concourse import bacc
    from concourse.bass import ts

    f32 = mybir.dt.float32
    bf16 = mybir.dt.bfloat16
    AF = mybir.ActivationFunctionType

    nc = bacc.Bacc(trn_type="TRN2", num_devices=num_devices, debug=False)

    # ---- per-core I/O ----
    xT = nc.dram_tensor("xT", [C, T], bf16, kind="ExternalInput")  # x[b].T
    wqkv = nc.dram_tensor("wqkv", [C, 3 * HG * D], bf16, kind="ExternalInput")
    bqk = nc.dram_tensor("bqk", [128, 16], f32, kind="ExternalInput")
    bv = nc.dram_tensor("bv", [1, HG * D], f32, kind="ExternalInput")
    # full-height rope tables: cos2 = [cos; cos], sinB = [sin; -sin]
    cosT = nc.dram_tensor("cosT", [D, T], bf16, kind="ExternalInput")
    sinT = nc.dram_tensor("sinT", [D, T], bf16, kind="ExternalInput")
    maskT = nc.dram_tensor("maskT", [128, 4, 512], bf16, kind="ExternalInput")
    wproj = nc.dram_tensor("wproj", [C, C // 2], bf16, kind="ExternalInput")
    bproj = nc.dram_tensor("bproj", [1, C // 2], f32, kind="ExternalInput")
    out = nc.dram_tensor("out", [T, C // 2], bf16, kind="ExternalOutput")

    xT_r = xT.ap().rearrange("(ct p) t -> p ct t", p=128)  # [128, 16, 1024]
    wqkv_r = wqkv.ap().rearrange("(ct p) j -> p ct j", p=128)  # [128, 16, 3072]
    wproj_r = wproj.ap().rearrange("(jt p) c -> p jt c", p=128)  # [128, 16, 1024]

    scale = 1.0 / math.sqrt(D)

    with tile.TileContext(nc) as tc:
        with (
            tc.tile_pool(name="const", bufs=1) as const,
            tc.tile_pool(name="persist", bufs=1) as persist,
            tc.tile_pool(name="wp_pool", bufs=1) as wp_pool,
            tc.tile_pool(name="ps_a", bufs=TUNE["ps_a"], space="PSUM") as psA,
            tc.tile_pool(name="ps_b", bufs=TUNE["ps_b"], space="PSUM") as psB,
            tc.tile_pool(name="ps_y", bufs=TUNE["ps_y"], space="PSUM") as psY,
            tc.tile_pool(name="ps_sum", bufs=TUNE["ps_s"], space="PSUM") as psS,
            tc.tile_pool(name="work", bufs=4) as work,
            tc.tile_pool(name="dram", bufs=1, space="DRAM") as drampool,
        ):
            # ---- persistent activations ----
            qf = persist.tile([128, HG, T], bf16)  # [d, h, t] rotated Q^T
            kf = persist.tile([128, HG, T], bf16)  # [d, h, t] rotated K^T
            v_all = persist.tile([128, 8, HG * D], bf16)  # [t_in, tt, j]
            yT = persist.tile([128, HG, T], bf16)  # [d, h, t] normalized att out

            # ---- DRAM staging for the collectives ----
            ybounce = drampool.tile([HG * D, T], bf16, name="ybounce")
            ygth_q = [
                drampool.tile([2 * len(hs) * D, T], bf16, name=f"ygth{w}")
                for w, hs in enumerate(AG_HEADS)
            ]
            yb_r = ybounce.rearrange("(h p) t -> p h t", p=128)

            def emit_ag(w):
                hs = AG_HEADS[w]
                rows = slice(D * hs[0], D * (hs[-1] + 1))
                nc.gpsimd.collective_compute(
                    "AllGather",
                    mybir.AluOpType.bypass,
                    replica_groups=[[0, 1], [2, 3], [4, 5], [6, 7]],
                    ins=[ybounce[rows, :].opt()],
                    outs=[ygth_q[w][:].opt()],
                )

            with (
                tc.tile_pool(name="xpool", bufs=1) as xpool,
                tc.tile_pool(name="wpool", bufs=2) as wpool,
                tc.tile_pool(name="rope", bufs=1) as rope_pool,
            ):
                xs = xpool.tile([128, 16, T], bf16, name="xs")
                wts = {}

                def load_chunk(chunk, eng, granules=1):
                    wt = wpool.tile([128, 16, 512], bf16, tag="wt", name="wt")
                    wts[chunk] = wt
                    cslice = slice(chunk * 512, (chunk + 1) * 512)
                    if granules == 1:
                        eng.dma_start(out=wt, in_=wqkv_r[:, :, cslice])
                    else:
                        step = 16 // granules
                        for g in range(granules):
                            cts = slice(g * step, (g + 1) * step)
                            eng.dma_start(
                                out=wt[:, cts, :], in_=wqkv_r[:, cts, cslice]
                            )

                # critical-path loads first: xs + chunk0 interleaved on sync,
                # biases/rope tables then chunk2/chunk4 on scalar
                wt0 = wpool.tile([128, 16, 512], bf16, tag="wt", name="wt")
                wts[0] = wt0
                for q in range(8):
                    cts = slice(2 * q, 2 * q + 2)
                    nc.sync.dma_start(out=xs[:, cts, :], in_=xT_r[:, cts, :])
                    nc.sync.dma_start(out=wt0[:, cts, :], in_=wqkv_r[:, cts, 0:512])
                bqk_sb = const.tile([128, 16], f32)
                nc.scalar.dma_start(out=bqk_sb, in_=bqk.ap())
                cos_sb = rope_pool.tile([128, T], bf16)
                nc.scalar.dma_start(out=cos_sb, in_=cosT.ap())
                sin_sb = rope_pool.tile([128, T], bf16)
                nc.scalar.dma_start(out=sin_sb, in_=sinT.ap())
                ones128 = const.tile([128, 128], bf16)
                nc.vector.memset(ones128, 1.0)
                # chunk2 behind xs/wt0 on sync (avoids racing them for HBM
                # bandwidth); chunk4 + consts on the scalar queue
                load_chunk(2, nc.sync, granules=4)
                load_chunk(4, nc.scalar, granules=2)
                mask_sb = const.tile([128, 4, 512], bf16)
                nc.scalar.dma_start(out=mask_sb, in_=maskT.ap())
                bv_bc = const.tile([128, HG * D], f32)
                nc.scalar.dma_start(
                    out=bv_bc, in_=bv.ap().to_broadcast([128, HG * D])
                )
                bp_bc = const.tile([128, C // 2], f32)
                nc.scalar.dma_start(
                    out=bp_bc, in_=bproj.ap().to_broadcast([128, C // 2])
                )

                wp = wp_pool.tile([128, 16, C // 2], bf16, name="wp")

                # ---------- emission generators ----------
                def qk_steps(chunk):
                    """Q or K projection + rope; yields mid-chain and at the
                    end so the filler granularity is 8 matmuls."""
                    wt = wts[chunk]
                    for jj in range(4):
                        jt = chunk * 4 + jj  # q: 0-7, k: 8-15
                        h = jt % 8
                        dest_all = qf if jt < 8 else kf
                        for th in range(2):
                            ps = psA.tile([128, 512], f32, tag="ps", name="ps")
                            for ct in range(16):
                                nc.tensor.matmul(
                                    ps,
                                    lhsT=wt[:, ct, jj * 128 : (jj + 1) * 128],
                                    rhs=xs[:, ct, ts(th, 512)],
                                    start=(ct == 0),
                                    stop=(ct == 15),
                                )
                                if ct == 7:
                                    yield
                            raw = work.tile(
                                [128, 512], bf16, tag="raw", name="raw", bufs=3
                            )
                            nc.scalar.activation(
                                raw, ps, AF.Identity,
                                bias=bqk_sb[:, jt : jt + 1],
                            )
                            dest = dest_all[:, h, ts(th, 512)]
                            # rotate-half on DVE: t2[0:64] = raw[64:]*(-sin),
                            # t2[64:] = raw[:64]*sin; dest = raw*cos + t2
                            t2 = work.tile(
                                [128, 512], bf16, tag="t2", name="t2", bufs=3
                            )
                            ss = ts(th, 512)
                            nc.vector.tensor_mul(
                                t2[0:64, :], raw[64:128, :], sin_sb[64:128, ss]
                            )
                            nc.vector.tensor_mul(
                                t2[64:128, :], raw[0:64, :], sin_sb[0:64, ss]
                            )
                            nc.vector.tensor_mul(dest, raw, cos_sb[:, ss])
                            nc.vector.tensor_add(dest, dest, t2)
                            yield

                def v_steps(chunk):
                    jc = chunk - 4  # 0 or 1
                    wt = wts[chunk]
                    for tt in range(8):
                        ps = psA.tile([128, 512], f32, tag="ps", name="ps")
                        for ct in range(16):
                            nc.tensor.matmul(
                                ps,
                                lhsT=xs[:, ct, ts(tt, 128)],
                                rhs=wt[:, ct, :],
                                start=(ct == 0),
                                stop=(ct == 15),
                            )
                            if ct == 7:
                                yield
                        nc.vector.tensor_add(
                            v_all[:, tt, jc * 512 : (jc + 1) * 512],
                            ps,
                            bv_bc[:, jc * 512 : (jc + 1) * 512],
                        )
                        yield

                ag_after = {1: 0, 3: 1, 5: 2, 6: 3, 7: 4}
                hooks = {"on_ag": None}  # set to ygs load_group in phase C

                def attn_steps(h):
                    """One yield between a block's score emission and its AV
                    matmul, so interleaved filler work hides the exp latency."""
                    for qc in range(2):
                        n_kt = 4 * (qc + 1)
                        ps_y = psY.tile([128, 512], f32, tag="ps_y", name="ps_y")
                        ps_sum = psS.tile(
                            [128, 512], f32, tag="ps_sum", name="ps_sum"
                        )
                        p_hold = None
                        padd_hold = None
                        for kt in range(n_kt):
                            ps_sc = psB.tile(
                                [128, 512], f32, tag="psb", name="ps_sc"
                            )
                            nc.tensor.matmul(
                                ps_sc,
                                lhsT=kf[:, h, ts(kt, 128)],
                                rhs=qf[:, h, ts(qc, 512)],
                                start=True,
                                stop=True,
                            )
                            p_sb = work.tile(
                                [128, 512], bf16, tag="p_sb", name="p_sb",
                                bufs=TUNE["p_sb_bufs"],
                            )
                            nc.scalar.activation(p_sb, ps_sc, AF.Exp, scale=scale)
                            kt_rel = kt - 4 * qc
                            if 0 <= kt_rel < 4:  # block straddles the diagonal
                                nc.vector.tensor_mul(
                                    p_sb, p_sb, mask_sb[:, kt_rel, :]
                                )
                            if kt % 2 == 0:
                                p_hold = p_sb
                            else:
                                padd = work.tile(
                                    [128, 512], bf16, tag="padd", name="padd",
                                    bufs=3,
                                )
                                nc.vector.tensor_add(padd, p_hold, p_sb)
                                if kt % 4 == 1:
                                    padd_hold = padd
                                else:
                                    pquad = work.tile(
                                        [128, 512], bf16, tag="pquad",
                                        name="pquad", bufs=2,
                                    )
                                    # off the DVE: GpSimd is otherwise idle
                                    nc.gpsimd.tensor_add(pquad, padd_hold, padd)
                                    # ones[128,128] stationary: column sums
                                    # land broadcast across all partitions
                                    nc.tensor.matmul(
                                        ps_sum,
                                        lhsT=ones128,
                                        rhs=pquad,
                                        start=(kt == 3),
                                        stop=(kt == n_kt - 1),
                                    )
                            yield
                            nc.tensor.matmul(
                                ps_y,
                                lhsT=v_all[:, kt, ts(h, 128)],
                                rhs=p_sb,
                                start=(kt == 0),
                                stop=(kt == n_kt - 1),
                            )
                        rb = work.tile(
                            [128, 512], f32, tag="rb", name="rb", bufs=2
                        )
                        nc.vector.reciprocal(rb, ps_sum)
                        nc.vector.tensor_mul(yT[:, h, ts(qc, 512)], ps_y, rb)
                        yield
                    nc.sync.dma_start(out=yb_r[:, h, :], in_=yT[:, h, :])
                    if h in ag_after:
                        emit_ag(ag_after[h])
                        if hooks["on_ag"] is not None:
                            hooks["on_ag"](ag_after[h])

                def chain(*gens):
                    for g in gens:
                        yield from g

                def drive(main, filler, per_step, lead=0):
                    """Advance `filler` per_step[i % len] times after each
                    main step; then drain both."""
                    for _ in range(lead):
                        if next(filler, None) is None:
                            break
                    i = 0
                    for _ in main:
                        for _ in range(per_step[i % len(per_step)]):
                            if next(filler, None) is None:
                                break
                        i += 1
                    for _ in filler:
                        pass

                # ========== phase A: chunks 0, 2, 4 (heads 0-3 + all V lo) ==
                for _ in chain(qk_steps(0), qk_steps(2), v_steps(4)):
                    pass

                # phase B loads on both queues, behind phase-A traffic
                load_chunk(1, nc.sync, granules=8)
                load_chunk(3, nc.scalar, granules=4)
                nc.sync.dma_start(out=wp, in_=wproj_r)
                load_chunk(5, nc.scalar, granules=2)

                # ========== phase B: chunks 1, 3, 5 ⊗ attention h0-3 ========
                drive(
                    chain(qk_steps(1), qk_steps(3), v_steps(5)),
                    chain(*[attn_steps(h) for h in range(4)]),
                    TUNE["attn_per_qkv"],
                )

            # ========== phase C: c_proj waves ⊗ attention h4-7 ==========
            # ygs slot j = global feature block j: slots 8g'+h come from AG
            # group of head h, rank g'.
            with (
                tc.tile_pool(name="stash_pool", bufs=1) as stash_pool,
                tc.tile_pool(name="ygs_pool", bufs=1) as ygs_pool,
            ):
                stash = stash_pool.tile([128, 16, 512], f32, name="stash")
                ygs = ygs_pool.tile([128, 16, T], bf16, name="ygs")

                def load_group(w):
                    hs = AG_HEADS[w]
                    ygth_r = ygth_q[w].rearrange("(j p) t -> p j t", p=128)
                    for r in range(2):
                        # split across the two HWDGE queues
                        eng = nc.scalar if r == 0 else nc.sync
                        for i, h in enumerate(hs):
                            eng.dma_start(
                                out=ygs[:, 8 * r + h, :],
                                in_=ygth_r[:, r * len(hs) + i, :],
                            )

                load_group(0)
                load_group(1)
                hooks["on_ag"] = load_group

                def proj_steps(heads, first, last, half=False):
                    """One c_proj tile per yield (mid-chain yield too when
                    `half`): chain over local+remote copies of `heads`, then
                    stash-accumulate (or final-merge + output DMA on `last`)."""
                    jts = [8 * r + h for h in heads for r in range(2)]
                    mid = len(jts) // 2
                    for tt in range(8):
                        for cc in range(2):
                            st = tt * 2 + cc
                            ps = psA.tile(
                                [128, 512], f32, tag="ps", name="ps_proj"
                            )
                            for i, jt in enumerate(jts):
                                if half and i == mid:
                                    yield
                                nc.tensor.matmul(
                                    ps,
                                    lhsT=ygs[:, jt, ts(tt, 128)],
                                    rhs=wp[:, jt, ts(cc, 512)],
                                    start=(i == 0),
                                    stop=(i == len(jts) - 1),
                                )
                            if first:
                                nc.vector.tensor_add(
                                    stash[:, st, :], ps, bp_bc[:, ts(cc, 512)]
                                )
                            elif not last:
                                nc.vector.tensor_add(
                                    stash[:, st, :], ps, stash[:, st, :]
                                )
                            else:
                                pb = work.tile(
                                    [128, 512], bf16, tag="pb", name="pb",
                                    bufs=4,
                                )
                                nc.vector.tensor_add(pb, ps, stash[:, st, :])
                                nc.sync.dma_start(
                                    out=out.ap()[ts(tt, 128), ts(cc, 512)],
                                    in_=pb,
                                )
                            yield

                # W0 {h0-3} ⊗ attn h4-6 at 4-matmul granularity;
                # W1 {h4,5} ⊗ attn h7; W2 {h6} covers AG(h7); W3 {h7} tail.
                drive(
                    proj_steps((0, 1, 2, 3), first=True, last=False, half=True),
                    chain(*[attn_steps(h) for h in (4, 5, 6)]),
                    (1,) * 15 + (2,),
                    lead=8,
                )
                drive(
                    proj_steps((4, 5), first=False, last=False),
                    attn_steps(7),
                    (1, 1, 1, 1, 1, 1, 1, 0),
                )
                for _ in proj_steps((6,), first=False, last=False):
                    pass
                for _ in proj_steps((7,), first=False, last=True):
                    pass

    # Pin every activation to the one table set holding Exp+Identity
    # (natural_log_exp_and_others) so the set-picker never inserts
    # ACT_TABLE_LOADs mid-kernel.
    import concourse.bacc as bacc_mod

    orig_tables = bacc_mod.get_activation_tables

    def _pinned_tables(arch):
        tabs = orig_tables(arch)
        return {
            name: (funcs if name == "natural_log_exp_and_others" else set())
            for name, funcs in tabs.items()
        }

    bacc_mod.get_activation_tables = _pinned_tables
    try:
        nc.finalize()
    finally:
        bacc_mod.get_activation_tables = orig_tables
    return nc


def _host_inputs(x, w_attn, b_attn, w_proj, b_proj):
    """Build the 8 per-core input maps."""
    x = np.asarray(x, np.float32)
    w_attn = np.asarray(w_attn, np.float32)
    b_attn = np.asarray(b_attn, np.float32)
    w_proj = np.asarray(w_proj, np.float32)
    b_proj = np.asarray(b_proj, np.float32)

    # rope tables, transposed [d, t], full height:
    # dest = raw * cos2 + t2 with sinB = [s; -s] read at the source window:
    #   t2[0:64] = raw[64:128] * sinB[64:128] = raw_hi * (-s)
    #   t2[64:128] = raw[0:64] * sinB[0:64]   = raw_lo * s
    inv_freq = 1.0 / (ROPE_BASE ** (np.arange(0, D, 2, dtype=np.float32) / D))
    freqs = np.arange(T, dtype=np.float32)[:, None] * inv_freq[None, :]  # [T, 64]
    c_ = np.ascontiguousarray(np.cos(freqs).T)  # [64, T]
    s_ = np.ascontiguousarray(np.sin(freqs).T)
    cosT = np.concatenate([c_, c_], axis=0).astype(BF16)  # [128, T]
    sinB = np.concatenate([s_, -s_], axis=0).astype(BF16)

    # causal mask blocks, transposed [k, q]: block kt_rel r, q chunk of 512
    k_idx = np.arange(128)
    q_idx = np.arange(512)
    maskT = np.zeros((128, 4, 512), np.float32)
    for r in range(4):
        maskT[:, r, :] = ((r * 128 + k_idx)[:, None] <= q_idx[None, :]).astype(
            np.float32
        )
    maskT = maskT.astype(BF16)

    in_maps = []
    for c in range(N_CORES):
        b, g = divmod(c, 2)
        cs = slice(g * 1024, (g + 1) * 1024)
        wq = w_attn[:, 0:C][:, cs]
        wk = w_attn[:, C : 2 * C][:, cs]
        wv = w_attn[:, 2 * C : 3 * C][:, cs]
        bq = b_attn[0:C][cs]
        bk = b_attn[C : 2 * C][cs]
        bvv = b_attn[2 * C : 3 * C][cs]
        in_maps.append(
            {
                "xT": np.ascontiguousarray(x[b].T).astype(BF16),
                "wqkv": np.concatenate([wq, wk, wv], axis=1).astype(BF16),
                "bqk": np.ascontiguousarray(
                    np.concatenate([bq, bk]).reshape(16, 128).T
                ).astype(np.float32),
                "bv": bvv.reshape(1, 1024).astype(np.float32),
                "cosT": cosT,
                "sinT": sinB,
                "maskT": maskT,
                "wproj": w_proj[:, cs].astype(BF16),
                "bproj": b_proj[cs].reshape(1, 1024).astype(np.float32),
            }
        )
    return in_maps


def kernel(x, w_attn, b_attn, w_proj, b_proj, _trace=False):
    from concourse.bass_utils import run_bass_kernel_spmd

    if "nc" not in _PROGRAM_CACHE:
        _PROGRAM_CACHE["nc"] = _build_program()
    nc = _PROGRAM_CACHE["nc"]

    in_maps = _host_inputs(x, w_attn, b_attn, w_proj, b_proj)
    res = run_bass_kernel_spmd(
        nc, in_maps, core_ids=list(range(N_CORES)), trace=_trace
    )
    _PROGRAM_CACHE["last_results"] = res

    out = np.zeros((B, T, C), np.float32)
    for c in range(N_CORES):
        b, g = divmod(c, 2)
        out[b, :, g * 1024 : (g + 1) * 1024] = np.asarray(
            res.results[c]["out"], dtype=np.float32
        )
    return out


# revision 21
# speedup vs baseline: 1.1935x; 1.0628x over previous
"""Causal self-attention (B=4, T=1024, C=2048, H=16, rotary) on 8 trn2 cores.

Sharding: core c = 2*b + g handles batch b, head-group g (heads 8g..8g+7).
 - QKV projection in transposed layout (Q^T/K^T = [d, T]; V natural [T, d]).
 - RoPE fully on DVE: rotate-half via 64-partition tensor ops with
   mismatched in/out partition windows (no PE permutation matmul).
 - Scores transposed S^T = K^T.Q -> [k, q]; softmax without max-subtraction;
   causal masking via multiplicative 0/1 bf16 masks on diagonal blocks.
 - Softmax denominator: pairwise adds (padd on DVE, pquad on GpSimd) then
   ones[128,128] matmul accumulates column sums broadcast across
   partitions; 1/d via DVE reciprocal (no scalar-engine Ln/Exp chain).
 - AllGather per head group as heads finish: groups (0,1),(2,3),(4,5),
   (6),(7) so the tail exchange is a single head, then c_proj in waves
   sized to cover each gather: W0{h0-3} interleaved with attention h4-6,
   W1{h4,5} with h7, then W2{h6}, W3{h7} (f32 SBUF stash accumulation,
   final merge -> bf16 -> output DMA per tile).
 - DMA issue is split across the two HWDGE queues (sync + scalar) so the
   QKV ramp is not serialized on one queue.
 - Emission interleaves attention blocks into the QKV / c_proj matmul
   streams at sub-microsecond granularity so the ACT-gated softmax never
   stalls the in-order PE queue.
All matmuls bf16 (fp32 PSUM accumulation).
"""

import math

import numpy as np
import ml_dtypes

BF16 = ml_dtypes.bfloat16

B, T, C = 4, 1024, 2048
H = 16  # total heads
D = C // H  # 128 head dim
HG = 8  # heads per group (per core)
N_CORES = 8
ROPE_BASE = 10000.0

TUNE = {
    "ps_a": 2,
    "ps_b": 2,
    "ps_y": 2,
    "ps_s": 2,
    "p_sb_bufs": 5,
    # 1 attn block between 8-matmul qkv half-steps (48 halves, 56 yields)
    "attn_per_qkv": (1, 1, 1, 1, 1, 2),
}

# AllGather groups of local heads; fired when the last head finishes.
AG_HEADS = ((0, 1), (2, 3), (4, 5), (6,), (7,))

_PROGRAM_CACHE = {}


def _build_program(num_devices=N_CORES):
    import concourse.mybir as mybir
    import concourse.tile as tile
    from concourse import bacc
    from concourse.bass import ts

    f32 = mybir.dt.float32
    bf16 = mybir.dt.bfloat16
    AF = mybir.ActivationFunctionType

    nc = bacc.Bacc(trn_type="TRN2", num_devices=num_devices, debug=False)

    # ---- per-core I/O ----
    xT = nc.dram_tensor("xT", [C, T], bf16, kind="ExternalInput")  # x[b].T
    wqkv = nc.dram_tensor("wqkv", [C, 3 * HG * D], bf16, kind="ExternalInput")
    bqk = nc.dram_tensor("bqk", [128, 16], f32, kind="ExternalInput")
    bv = nc.dram_tensor("bv", [1, HG * D], f32, kind="ExternalInput")
    # full-height rope tables: cos2 = [cos; cos], sinB = [sin; -sin]
    cosT = nc.dram_tensor("cosT", [D, T], bf16, kind="ExternalInput")
    sinT = nc.dram_tensor("sinT", [D, T], bf16, kind="ExternalInput")
    maskT = nc.dram_tensor("maskT", [128, 4, 512], bf16, kind="ExternalInput")
    wproj = nc.dram_tensor("wproj", [C, C // 2], bf16, kind="ExternalInput")
    bproj = nc.dram_tensor("bproj", [1, C // 2], f32, kind="ExternalInput")
    out = nc.dram_tensor("out", [T, C // 2], bf16, kind="ExternalOutput")

    xT_r = xT.ap().rearrange("(ct p) t -> p ct t", p=128)  # [128, 16, 1024]
    wqkv_r = wqkv.ap().rearrange("(ct p) j -> p ct j", p=128)  # [128, 16, 3072]
    wproj_r = wproj.ap().rearrange("(jt p) c -> p jt c", p=128)  # [128, 16, 1024]

    scale = 1.0 / math.sqrt(D)

    with tile.TileContext(nc) as tc:
        with (
            tc.tile_pool(name="const", bufs=1) as const,
            tc.tile_pool(name="persist", bufs=1) as persist,
            tc.tile_pool(name="wp_pool", bufs=1) as wp_pool,
            tc.tile_pool(name="ps_a", bufs=TUNE["ps_a"], space="PSUM") as psA,
            tc.tile_pool(name="ps_b", bufs=TUNE["ps_b"], space="PSUM") as psB,
            tc.tile_pool(name="ps_y", bufs=TUNE["ps_y"], space="PSUM") as psY,
            tc.tile_pool(name="ps_sum", bufs=TUNE["ps_s"], space="PSUM") as psS,
            tc.tile_pool(name="work", bufs=4) as work,
            tc.tile_pool(name="dram", bufs=1, space="DRAM") as drampool,
        ):
            # ---- persistent activations ----
            qf = persist.tile([128, HG, T], bf16)  # [d, h, t] rotated Q^T
            kf = persist.tile([128, HG, T], bf16)  # [d, h, t] rotated K^T
            v_all = persist.tile([128, 8, HG * D], bf16)  # [t_in, tt, j]
            yT = persist.tile([128, HG, T], bf16)  # [d, h, t] normalized att out

            # ---- DRAM staging for the collectives ----
            ybounce = drampool.tile([HG * D, T], bf16, name="ybounce")
            ygth_q = [
                drampool.tile([2 * len(hs) * D, T], bf16, name=f"ygth{w}")
                for w, hs in enumerate(AG_HEADS)
            ]
            yb_r = ybounce.rearrange("(h p) t -> p h t", p=128)

            def emit_ag(w):
                hs = AG_HEADS[w]
                rows = slice(D * hs[0], D * (hs[-1] + 1))
                nc.gpsimd.collective_compute(
                    "AllGather",
                    mybir.AluOpType.bypass,
                    replica_groups=[[0, 1], [2, 3], [4, 5], [6, 7]],
                    ins=[ybounce[rows, :].opt()],
                    outs=[ygth_q[w][:].opt()],
                )

            with (
                tc.tile_pool(name="xpool", bufs=1) as xpool,
                tc.tile_pool(name="wpool", bufs=2) as wpool,
                tc.tile_pool(name="rope", bufs=1) as rope_pool,
            ):
                xs = xpool.tile([128, 16, T], bf16, name="xs")
                wts = {}

                def load_chunk(chunk, eng, granules=1):
                    wt = wpool.tile([128, 16, 512], bf16, tag="wt", name="wt")
                    wts[chunk] = wt
                    cslice = slice(chunk * 512, (chunk + 1) * 512)
                    if granules == 1:
                        eng.dma_start(out=wt, in_=wqkv_r[:, :, cslice])
                    else:
                        step = 16 // granules
                        for g in range(granules):
                            cts = slice(g * step, (g + 1) * step)
                            eng.dma_start(
                                out=wt[:, cts, :], in_=wqkv_r[:, cts, cslice]
                            )

                # critical-path loads first: xs + chunk0 interleaved on sync,
                # biases/rope tables then chunk2/chunk4 on scalar
                wt0 = wpool.tile([128, 16, 512], bf16, tag="wt", name="wt")
                wts[0] = wt0
                for q in range(8):
                    cts = slice(2 * q, 2 * q + 2)
                    nc.sync.dma_start(out=xs[:, cts, :], in_=xT_r[:, cts, :])
                    nc.sync.dma_start(out=wt0[:, cts, :], in_=wqkv_r[:, cts, 0:512])
                bqk_sb = const.tile([128, 16], f32)
                nc.scalar.dma_start(out=bqk_sb, in_=bqk.ap())
                cos_sb = rope_pool.tile([128, T], bf16)
                nc.scalar.dma_start(out=cos_sb, in_=cosT.ap())
                sin_sb = rope_pool.tile([128, T], bf16)
                nc.scalar.dma_start(out=sin_sb, in_=sinT.ap())
                ones128 = const.tile([128, 128], bf16)
                nc.vector.memset(ones128, 1.0)
                # chunk2 behind xs/wt0 on sync (avoids racing them for HBM
                # bandwidth); chunk4 + consts on the scalar queue
                load_chunk(2, nc.sync, granules=4)
                load_chunk(4, nc.scalar, granules=2)
                mask_sb = const.tile([128, 4, 512], bf16)
                nc.scalar.dma_start(out=mask_sb, in_=maskT.ap())
                bv_bc = const.tile([128, HG * D], f32)
                nc.scalar.dma_start(
                    out=bv_bc, in_=bv.ap().to_broadcast([128, HG * D])
                )
                bp_bc = const.tile([128, C // 2], f32)
                nc.scalar.dma_start(
                    out=bp_bc, in_=bproj.ap().to_broadcast([128, C // 2])
                )

                wp = wp_pool.tile([128, 16, C // 2], bf16, name="wp")

                # ---------- emission generators ----------
                def qk_steps(chunk):
                    """Q or K projection + rope; yields mid-chain and at the
                    end so the filler granularity is 8 matmuls."""
                    wt = wts[chunk]
                    for jj in range(4):
                        jt = chunk * 4 + jj  # q: 0-7, k: 8-15
                        h = jt % 8
                        dest_all = qf if jt < 8 else kf
                        for th in range(2):
                            ps = psA.tile([128, 512], f32, tag="ps", name="ps")
                            for ct in range(16):
                                nc.tensor.matmul(
                                    ps,
                                    lhsT=wt[:, ct, jj * 128 : (jj + 1) * 128],
                                    rhs=xs[:, ct, ts(th, 512)],
                                    start=(ct == 0),
                                    stop=(ct == 15),
                                )
                                if ct == 7:
                                    yield
                            raw = work.tile(
                                [128, 512], bf16, tag="raw", name="raw", bufs=3
                            )
                            nc.scalar.activation(
                                raw, ps, AF.Identity,
                                bias=bqk_sb[:, jt : jt + 1],
                            )
                            dest = dest_all[:, h, ts(th, 512)]
                            # rotate-half on DVE: t2[0:64] = raw[64:]*(-sin),
                            # t2[64:] = raw[:64]*sin; dest = raw*cos + t2
                            t2 = work.tile(
                                [128, 512], bf16, tag="t2", name="t2", bufs=3
                            )
                            ss = ts(th, 512)
                            nc.vector.tensor_mul(
                                t2[0:64, :], raw[64:128, :], sin_sb[64:128, ss]
                            )
                            nc.vector.tensor_mul(
                                t2[64:128, :], raw[0:64, :], sin_sb[0:64, ss]
                            )
                            nc.vector.tensor_mul(dest, raw, cos_sb[:, ss])
                            nc.vector.tensor_add(dest, dest, t2)
                            yield

                def v_steps(chunk):
                    jc = chunk - 4  # 0 or 1
                    wt = wts[chunk]
                    for tt in range(8):
                        ps = psA.tile([128, 512], f32, tag="ps", name="ps")
                        for ct in range(16):
                            nc.tensor.matmul(
                                ps,
                                lhsT=xs[:, ct, ts(tt, 128)],
                                rhs=wt[:, ct, :],
                                start=(ct == 0),
                                stop=(ct == 15),
                            )
                            if ct == 7:
                                yield
                        nc.vector.tensor_add(
                            v_all[:, tt, jc * 512 : (jc + 1) * 512],
                            ps,
                            bv_bc[:, jc * 512 : (jc + 1) * 512],
                        )
                        yield

                ag_after = {1: 0, 3: 1, 5: 2, 6: 3, 7: 4}
                hooks = {"on_ag": None}  # set to ygs load_group in phase C

                def attn_steps(h):
                    """One yield between a block's score emission and its AV
                    matmul, so interleaved filler work hides the exp latency."""
                    for qc in range(2):
                        n_kt = 4 * (qc + 1)
                        ps_y = psY.tile([128, 512], f32, tag="ps_y", name="ps_y")
                        ps_sum = psS.tile(
                            [128, 512], f32, tag="ps_sum", name="ps_sum"
                        )
                        p_hold = None
                        padd_hold = None
                        for kt in range(n_kt):
                            ps_sc = psB.tile(
                                [128, 512], f32, tag="psb", name="ps_sc"
                            )
                            nc.tensor.matmul(
                                ps_sc,
                                lhsT=kf[:, h, ts(kt, 128)],
                                rhs=qf[:, h, ts(qc, 512)],
                                start=True,
                                stop=True,
                            )
                            p_sb = work.tile(
                                [128, 512], bf16, tag="p_sb", name="p_sb",
                                bufs=TUNE["p_sb_bufs"],
                            )
                            nc.scalar.activation(p_sb, ps_sc, AF.Exp, scale=scale)
                            kt_rel = kt - 4 * qc
                            if 0 <= kt_rel < 4:  # block straddles the diagonal
                                nc.vector.tensor_mul(
                                    p_sb, p_sb, mask_sb[:, kt_rel, :]
                                )
                            if kt % 2 == 0:
                                p_hold = p_sb
                            else:
                                padd = work.tile(
                                    [128, 512], bf16, tag="padd", name="padd",
                                    bufs=3,
                                )
                                nc.vector.tensor_add(padd, p_hold, p_sb)
                                if kt % 4 == 1:
                                    padd_hold = padd
                                else:
                                    pquad = work.tile(
                                        [128, 512], bf16, tag="pquad",
                                        name="pquad", bufs=2,
                                    )
                                    # off the DVE: GpSimd is otherwise idle
                                    nc.gpsimd.tensor_add(pquad, padd_hold, padd)
                                    # ones[128,128] stationary: column sums
                                    # land broadcast across all partitions
                                    nc.tensor.matmul(
                                        ps_sum,
                                        lhsT=ones128,
                                        rhs=pquad,
                                        start=(kt == 3),
                                        stop=(kt == n_kt - 1),
                                    )
                            yield
                            nc.tensor.matmul(
                                ps_y,
                                lhsT=v_all[:, kt, ts(h, 128)],
                                rhs=p_sb,
                                start=(kt == 0),
                                stop=(kt == n_kt - 1),
                            )
                        # 1/denom = exp(-ln(denom)); Ln/Exp share a table set
                        # (DVE reciprocal measures ~3.4us — far too slow)
                        lnt = work.tile(
                            [128, 512], f32, tag="lnt", name="lnt", bufs=2
                        )
                        nc.scalar.activation(lnt, ps_sum, AF.Ln)
                        rb = work.tile(
                            [128, 512], bf16, tag="rb", name="rb", bufs=2
                        )
                        nc.scalar.activation(rb, lnt, AF.Exp, scale=-1.0)
                        nc.vector.tensor_mul(yT[:, h, ts(qc, 512)], ps_y, rb)
                        yield
                    nc.sync.dma_start(out=yb_r[:, h, :], in_=yT[:, h, :])
                    if h in ag_after:
                        emit_ag(ag_after[h])
                        if hooks["on_ag"] is not None:
                            hooks["on_ag"](ag_after[h])

                def chain(*gens):
                    for g in gens:
                        yield from g

                def drive(main, filler, per_step, lead=0):
                    """Advance `filler` per_step[i % len] times after each
                    main step; then drain both."""
                    for _ in range(lead):
                        if next(filler, None) is None:
                            break
                    i = 0
                    for _ in main:
                        for _ in range(per_step[i % len(per_step)]):
                            if next(filler, None) is None:
                                break
                        i += 1
                    for _ in filler:
                        pass

                # ========== phase A: chunks 0, 2, 4 (heads 0-3 + all V lo) ==
                for _ in chain(qk_steps(0), qk_steps(2), v_steps(4)):
                    pass

                # phase B loads on both queues, behind phase-A traffic
                load_chunk(1, nc.sync, granules=8)
                load_chunk(3, nc.scalar, granules=4)
                nc.sync.dma_start(out=wp, in_=wproj_r)
                load_chunk(5, nc.scalar, granules=2)

                # ========== phase B: chunks 1, 3, 5 ⊗ attention h0-3 ========
                drive(
                    chain(qk_steps(1), qk_steps(3), v_steps(5)),
                    chain(*[attn_steps(h) for h in range(4)]),
                    TUNE["attn_per_qkv"],
                )

            # ========== phase C: c_proj waves ⊗ attention h4-7 ==========
            # ygs slot j = global feature block j: slots 8g'+h come from AG
            # group of head h, rank g'.
            with (
                tc.tile_pool(name="stash_pool", bufs=1) as stash_pool,
                tc.tile_pool(name="ygs_pool", bufs=1) as ygs_pool,
            ):
                stash = stash_pool.tile([128, 16, 512], f32, name="stash")
                ygs = ygs_pool.tile([128, 16, T], bf16, name="ygs")

                def load_group(w):
                    hs = AG_HEADS[w]
                    ygth_r = ygth_q[w].rearrange("(j p) t -> p j t", p=128)
                    for r in range(2):
                        # split across the two HWDGE queues
                        eng = nc.scalar if r == 0 else nc.sync
                        for i, h in enumerate(hs):
                            eng.dma_start(
                                out=ygs[:, 8 * r + h, :],
                                in_=ygth_r[:, r * len(hs) + i, :],
                            )

                load_group(0)
                load_group(1)
                hooks["on_ag"] = load_group

                def proj_steps(heads, first, last, half=False):
                    """One c_proj tile per yield (mid-chain yield too when
                    `half`): chain over local+remote copies of `heads`, then
                    stash-accumulate (or final-merge + output DMA on `last`)."""
                    jts = [8 * r + h for h in heads for r in range(2)]
                    mid = len(jts) // 2
                    for tt in range(8):
                        for cc in range(2):
                            st = tt * 2 + cc
                            ps = psA.tile(
                                [128, 512], f32, tag="ps", name="ps_proj"
                            )
                            for i, jt in enumerate(jts):
                                if half and i == mid:
                                    yield
                                nc.tensor.matmul(
                                    ps,
                                    lhsT=ygs[:, jt, ts(tt, 128)],
                                    rhs=wp[:, jt, ts(cc, 512)],
                                    start=(i == 0),
                                    stop=(i == len(jts) - 1),
                                )
                            if first:
                                nc.vector.tensor_add(
                                    stash[:, st, :], ps, bp_bc[:, ts(cc, 512)]
                                )
                            elif not last:
                                nc.vector.tensor_add(
                                    stash[:, st, :], ps, stash[:, st, :]
                                )
                            else:
                                pb = work.tile(
                                    [128, 512], bf16, tag="pb", name="pb",
                                    bufs=4,
                                )
                                nc.vector.tensor_add(pb, ps, stash[:, st, :])
                                nc.sync.dma_start(
                                    out=out.ap()[ts(tt, 128), ts(cc, 512)],
                                    in_=pb,
                                )
                            yield

                # W0 {h0-3} ⊗ attn h4-6 at 4-matmul granularity;
                # W1 {h4,5} ⊗ attn h7; W2 {h6} covers AG(h7); W3 {h7} tail.
                drive(
                    proj_steps((0, 1, 2, 3), first=True, last=False, half=True),
                    chain(*[attn_steps(h) for h in (4, 5, 6)]),
                    (1,) * 15 + (2,),
                    lead=8,
                )
                drive(
                    proj_steps((4, 5), first=False, last=False),
                    attn_steps(7),
                    (1, 1, 1, 1, 1, 1, 1, 0),
                )
                for _ in proj_steps((6,), first=False, last=False):
                    pass
                for _ in proj_steps((7,), first=False, last=True):
                    pass

    # Pin every activation to the one table set holding Exp+Identity
    # (natural_log_exp_and_others) so the set-picker never inserts
    # ACT_TABLE_LOADs mid-kernel.
    import concourse.bacc as bacc_mod

    orig_tables = bacc_mod.get_activation_tables

    def _pinned_tables(arch):
        tabs = orig_tables(arch)
        return {
            name: (funcs if name == "natural_log_exp_and_others" else set())
            for name, funcs in tabs.items()
        }

    bacc_mod.get_activation_tables = _pinned_tables
    try:
        nc.finalize()
    finally:
        bacc_mod.get_activation_tables = orig_tables
    return nc


def _host_inputs(x, w_attn, b_attn, w_proj, b_proj):
    """Build the 8 per-core input maps."""
    x = np.asarray(x, np.float32)
    w_attn = np.asarray(w_attn, np.float32)
    b_attn = np.asarray(b_attn, np.float32)
    w_proj = np.asarray(w_proj, np.float32)
    b_proj = np.asarray(b_proj, np.float32)

    # rope tables, transposed [d, t], full height:
    # dest = raw * cos2 + t2 with sinB = [s; -s] read at the source window:
    #   t2[0:64] = raw[64:128] * sinB[64:128] = raw_hi * (-s)
    #   t2[64:128] = raw[0:64] * sinB[0:64]   = raw_lo * s
    inv_freq = 1.0 / (ROPE_BASE ** (np.arange(0, D, 2, dtype=np.float32) / D))
    freqs = np.arange(T, dtype=np.float32)[:, None] * inv_freq[None, :]  # [T, 64]
    c_ = np.ascontiguousarray(np.cos(freqs).T)  # [64, T]
    s_ = np.ascontiguousarray(np.sin(freqs).T)
    cosT = np.concatenate([c_, c_], axis=0).astype(BF16)  # [128, T]
    sinB = np.concatenate([s_, -s_], axis=0).astype(BF16)

    # causal mask blocks, transposed [k, q]: block kt_rel r, q chunk of 512
    k_idx = np.arange(128)
    q_idx = np.arange(512)
    maskT = np.zeros((128, 4, 512), np.float32)
    for r in range(4):
        maskT[:, r, :] = ((r * 128 + k_idx)[:, None] <= q_idx[None, :]).astype(
            np.float32
        )
    maskT = maskT.astype(BF16)

    in_maps = []
    for c in range(N_CORES):
        b, g = divmod(c, 2)
        cs = slice(g * 1024, (g + 1) * 1024)
        wq = w_attn[:, 0:C][:, cs]
        wk = w_attn[:, C : 2 * C][:, cs]
        wv = w_attn[:, 2 * C : 3 * C][:, cs]
        bq = b_attn[0:C][cs]
        bk = b_attn[C : 2 * C][cs]
        bvv = b_attn[2 * C : 3 * C][cs]
        in_maps.append(
            {
                "xT": np.ascontiguousarray(x[b].T).astype(BF16),
                "wqkv": np.concatenate([wq, wk, wv], axis=1).astype(BF16),
                "bqk": np.ascontiguousarray(
                    np.concatenate([bq, bk]).reshape(16, 128).T
                ).astype(np.float32),
                "bv": bvv.reshape(1, 1024).astype(np.float32),
                "cosT": cosT,
                "sinT": sinB,
                "maskT": maskT,
                "wproj": w_proj[:, cs].astype(BF16),
                "bproj": b_proj[cs].reshape(1, 1024).astype(np.float32),
            }
        )
    return in_maps


def kernel(x, w_attn, b_attn, w_proj, b_proj, _trace=False):
    from concourse.bass_utils import run_bass_kernel_spmd

    if "nc" not in _PROGRAM_CACHE:
        _PROGRAM_CACHE["nc"] = _build_program()
    nc = _PROGRAM_CACHE["nc"]

    in_maps = _host_inputs(x, w_attn, b_attn, w_proj, b_proj)
    res = run_bass_kernel_spmd(
        nc, in_maps, core_ids=list(range(N_CORES)), trace=_trace
    )
    _PROGRAM_CACHE["last_results"] = res

    out = np.zeros((B, T, C), np.float32)
    for c in range(N_CORES):
        b, g = divmod(c, 2)
        out[b, :, g * 1024 : (g + 1) * 1024] = np.asarray(
            res.results[c]["out"], dtype=np.float32
        )
    return out


# revision 24
# speedup vs baseline: 1.3023x; 1.0911x over previous
"""Causal self-attention (B=4, T=1024, C=2048, H=16, rotary) on 8 trn2 cores.

Sharding: core c = 2*b + g handles batch b, head-group g (heads 8g..8g+7).
 - QKV projection in transposed layout (Q^T/K^T = [d, T]; V natural [T, d]).
 - RoPE fully on DVE: rotate-half via 64-partition tensor ops with
   mismatched in/out partition windows (no PE permutation matmul).
 - Scores transposed S^T = K^T.Q -> [k, q]; softmax without max-subtraction;
   causal masking via multiplicative 0/1 bf16 masks on diagonal blocks.
 - Softmax denominator: pairwise adds (padd on DVE, pquad on GpSimd) then
   ones[128,128] matmul accumulates column sums broadcast across
   partitions; 1/d via DVE reciprocal (no scalar-engine Ln/Exp chain).
 - AllGather per head group as heads finish: groups (0,1),(2,3),(4,5),
   (6),(7) so the tail exchange is a single head, then c_proj in waves
   sized to cover each gather: W0{h0-3} interleaved with attention h4-6,
   W1{h4,5} with h7, then W2{h6}, W3{h7} (f32 SBUF stash accumulation,
   final merge -> bf16 -> output DMA per tile).
 - DMA issue is split across the two HWDGE queues (sync + scalar) so the
   QKV ramp is not serialized on one queue.
 - Emission interleaves attention blocks into the QKV / c_proj matmul
   streams at sub-microsecond granularity so the ACT-gated softmax never
   stalls the in-order PE queue.
All matmuls bf16 (fp32 PSUM accumulation).
"""

import math

import numpy as np
import ml_dtypes

BF16 = ml_dtypes.bfloat16

B, T, C = 4, 1024, 2048
H = 16  # total heads
D = C // H  # 128 head dim
HG = 8  # heads per group (per core)
N_CORES = 8
ROPE_BASE = 10000.0

TUNE = {
    "ps_a": 2,
    "ps_b": 2,
    "ps_y": 2,
    "ps_s": 2,
    "p_sb_bufs": 5,
    # front-loaded: 56 attn yields done by quantum ~38 of 48, so h3's
    # AllGather fires well before phase C needs its ygs slots
    "attn_per_qkv": (2, 1),
}

# AllGather groups of local heads; fired when the last head finishes.
AG_HEADS = ((0, 1), (2, 3), (4, 5), (6,), (7,))

_PROGRAM_CACHE = {}


def _build_program(num_devices=N_CORES):
    import concourse.mybir as mybir
    import concourse.tile as tile
    from concourse import bacc
    from concourse.bass import ts

    f32 = mybir.dt.float32
    bf16 = mybir.dt.bfloat16
    AF = mybir.ActivationFunctionType

    nc = bacc.Bacc(trn_type="TRN2", num_devices=num_devices, debug=False)

    # ---- per-core I/O ----
    xT = nc.dram_tensor("xT", [C, T], bf16, kind="ExternalInput")  # x[b].T
    wqkv = nc.dram_tensor("wqkv", [C, 3 * HG * D], bf16, kind="ExternalInput")
    bqk = nc.dram_tensor("bqk", [128, 16], f32, kind="ExternalInput")
    bv = nc.dram_tensor("bv", [1, HG * D], f32, kind="ExternalInput")
    # full-height rope tables: cos2 = [cos; cos], sinB = [sin; -sin]
    cosT = nc.dram_tensor("cosT", [D, T], bf16, kind="ExternalInput")
    sinT = nc.dram_tensor("sinT", [D, T], bf16, kind="ExternalInput")
    maskT = nc.dram_tensor("maskT", [128, 4, 512], bf16, kind="ExternalInput")
    wproj = nc.dram_tensor("wproj", [C, C // 2], bf16, kind="ExternalInput")
    bproj = nc.dram_tensor("bproj", [1, C // 2], f32, kind="ExternalInput")
    out = nc.dram_tensor("out", [T, C // 2], bf16, kind="ExternalOutput")

    xT_r = xT.ap().rearrange("(ct p) t -> p ct t", p=128)  # [128, 16, 1024]
    wqkv_r = wqkv.ap().rearrange("(ct p) j -> p ct j", p=128)  # [128, 16, 3072]
    wproj_r = wproj.ap().rearrange("(jt p) c -> p jt c", p=128)  # [128, 16, 1024]

    scale = 1.0 / math.sqrt(D)

    with tile.TileContext(nc) as tc:
        with (
            tc.tile_pool(name="const", bufs=1) as const,
            tc.tile_pool(name="persist", bufs=1) as persist,
            tc.tile_pool(name="wp_pool", bufs=1) as wp_pool,
            tc.tile_pool(name="ps_a", bufs=TUNE["ps_a"], space="PSUM") as psA,
            tc.tile_pool(name="ps_b", bufs=TUNE["ps_b"], space="PSUM") as psB,
            tc.tile_pool(name="ps_y", bufs=TUNE["ps_y"], space="PSUM") as psY,
            tc.tile_pool(name="ps_sum", bufs=TUNE["ps_s"], space="PSUM") as psS,
            tc.tile_pool(name="work", bufs=4) as work,
            tc.tile_pool(name="dram", bufs=1, space="DRAM") as drampool,
        ):
            # ---- persistent activations ----
            qf = persist.tile([128, HG, T], bf16)  # [d, h, t] rotated Q^T
            kf = persist.tile([128, HG, T], bf16)  # [d, h, t] rotated K^T
            v_all = persist.tile([128, 8, HG * D], bf16)  # [t_in, tt, j]
            yT = persist.tile([128, HG, T], bf16)  # [d, h, t] normalized att out

            # ---- DRAM staging for the collectives ----
            ybounce = drampool.tile([HG * D, T], bf16, name="ybounce")
            ygth_q = [
                drampool.tile([2 * len(hs) * D, T], bf16, name=f"ygth{w}")
                for w, hs in enumerate(AG_HEADS)
            ]
            yb_r = ybounce.rearrange("(h p) t -> p h t", p=128)

            def emit_ag(w):
                hs = AG_HEADS[w]
                rows = slice(D * hs[0], D * (hs[-1] + 1))
                nc.gpsimd.collective_compute(
                    "AllGather",
                    mybir.AluOpType.bypass,
                    replica_groups=[[0, 1], [2, 3], [4, 5], [6, 7]],
                    ins=[ybounce[rows, :].opt()],
                    outs=[ygth_q[w][:].opt()],
                )

            with (
                tc.tile_pool(name="xpool", bufs=1) as xpool,
                tc.tile_pool(name="wpool", bufs=2) as wpool,
                tc.tile_pool(name="rope", bufs=1) as rope_pool,
            ):
                xs = xpool.tile([128, 16, T], bf16, name="xs")
                wts = {}

                def load_chunk(chunk, eng, granules=1):
                    wt = wpool.tile([128, 16, 512], bf16, tag="wt", name="wt")
                    wts[chunk] = wt
                    cslice = slice(chunk * 512, (chunk + 1) * 512)
                    if granules == 1:
                        eng.dma_start(out=wt, in_=wqkv_r[:, :, cslice])
                    else:
                        step = 16 // granules
                        for g in range(granules):
                            cts = slice(g * step, (g + 1) * step)
                            eng.dma_start(
                                out=wt[:, cts, :], in_=wqkv_r[:, cts, cslice]
                            )

                # critical-path loads first: xs + chunk0 interleaved on sync,
                # biases/rope tables then chunk2/chunk4 on scalar
                wt0 = wpool.tile([128, 16, 512], bf16, tag="wt", name="wt")
                wts[0] = wt0
                for q in range(8):
                    cts = slice(2 * q, 2 * q + 2)
                    nc.sync.dma_start(out=xs[:, cts, :], in_=xT_r[:, cts, :])
                    nc.sync.dma_start(out=wt0[:, cts, :], in_=wqkv_r[:, cts, 0:512])
                bqk_sb = const.tile([128, 16], f32)
                nc.scalar.dma_start(out=bqk_sb, in_=bqk.ap())
                cos_sb = rope_pool.tile([128, T], bf16)
                nc.scalar.dma_start(out=cos_sb, in_=cosT.ap())
                sin_sb = rope_pool.tile([128, T], bf16)
                nc.scalar.dma_start(out=sin_sb, in_=sinT.ap())
                ones128 = const.tile([128, 128], bf16)
                nc.vector.memset(ones128, 1.0)
                # chunk2 behind xs/wt0 on sync (avoids racing them for HBM
                # bandwidth); chunk4 + consts on the scalar queue
                load_chunk(2, nc.sync, granules=4)
                load_chunk(4, nc.scalar, granules=2)
                mask_sb = const.tile([128, 4, 512], bf16)
                nc.scalar.dma_start(out=mask_sb, in_=maskT.ap())
                bv_bc = const.tile([128, HG * D], f32)
                nc.scalar.dma_start(
                    out=bv_bc, in_=bv.ap().to_broadcast([128, HG * D])
                )
                bp_bc = const.tile([128, C // 2], f32)
                nc.scalar.dma_start(
                    out=bp_bc, in_=bproj.ap().to_broadcast([128, C // 2])
                )

                wp = wp_pool.tile([128, 16, C // 2], bf16, name="wp")

                # ---------- emission generators ----------
                def qk_steps(chunk):
                    """Q or K projection + rope; yields mid-chain and at the
                    end so the filler granularity is 8 matmuls."""
                    wt = wts[chunk]
                    for jj in range(4):
                        jt = chunk * 4 + jj  # q: 0-7, k: 8-15
                        h = jt % 8
                        dest_all = qf if jt < 8 else kf
                        for th in range(2):
                            ps = psA.tile([128, 512], f32, tag="ps", name="ps")
                            for ct in range(16):
                                nc.tensor.matmul(
                                    ps,
                                    lhsT=wt[:, ct, jj * 128 : (jj + 1) * 128],
                                    rhs=xs[:, ct, ts(th, 512)],
                                    start=(ct == 0),
                                    stop=(ct == 15),
                                )
                                if ct == 7:
                                    yield
                            raw = work.tile(
                                [128, 512], bf16, tag="raw", name="raw", bufs=3
                            )
                            nc.scalar.activation(
                                raw, ps, AF.Identity,
                                bias=bqk_sb[:, jt : jt + 1],
                            )
                            dest = dest_all[:, h, ts(th, 512)]
                            # rotate-half on DVE: t2[0:64] = raw[64:]*(-sin),
                            # t2[64:] = raw[:64]*sin; dest = raw*cos + t2
                            t2 = work.tile(
                                [128, 512], bf16, tag="t2", name="t2", bufs=3
                            )
                            ss = ts(th, 512)
                            nc.vector.tensor_mul(
                                t2[0:64, :], raw[64:128, :], sin_sb[64:128, ss]
                            )
                            nc.vector.tensor_mul(
                                t2[64:128, :], raw[0:64, :], sin_sb[0:64, ss]
                            )
                            nc.vector.tensor_mul(dest, raw, cos_sb[:, ss])
                            nc.vector.tensor_add(dest, dest, t2)
                            yield

                def v_steps(chunk):
                    jc = chunk - 4  # 0 or 1
                    wt = wts[chunk]
                    for tt in range(8):
                        ps = psA.tile([128, 512], f32, tag="ps", name="ps")
                        for ct in range(16):
                            nc.tensor.matmul(
                                ps,
                                lhsT=xs[:, ct, ts(tt, 128)],
                                rhs=wt[:, ct, :],
                                start=(ct == 0),
                                stop=(ct == 15),
                            )
                            if ct == 7:
                                yield
                        nc.vector.tensor_add(
                            v_all[:, tt, jc * 512 : (jc + 1) * 512],
                            ps,
                            bv_bc[:, jc * 512 : (jc + 1) * 512],
                        )
                        yield

                ag_after = {1: 0, 3: 1, 5: 2, 6: 3, 7: 4}
                hooks = {"on_ag": None}  # set to ygs load_group in phase C

                def attn_steps(h):
                    """One yield between a block's score emission and its AV
                    matmul, so interleaved filler work hides the exp latency."""
                    for qc in range(2):
                        n_kt = 4 * (qc + 1)
                        ps_y = psY.tile([128, 512], f32, tag="ps_y", name="ps_y")
                        ps_sum = psS.tile(
                            [128, 512], f32, tag="ps_sum", name="ps_sum"
                        )
                        p_hold = None
                        padd_hold = None
                        for kt in range(n_kt):
                            ps_sc = psB.tile(
                                [128, 512], f32, tag="psb", name="ps_sc"
                            )
                            nc.tensor.matmul(
                                ps_sc,
                                lhsT=kf[:, h, ts(kt, 128)],
                                rhs=qf[:, h, ts(qc, 512)],
                                start=True,
                                stop=True,
                            )
                            p_sb = work.tile(
                                [128, 512], bf16, tag="p_sb", name="p_sb",
                                bufs=TUNE["p_sb_bufs"],
                            )
                            nc.scalar.activation(p_sb, ps_sc, AF.Exp, scale=scale)
                            kt_rel = kt - 4 * qc
                            if 0 <= kt_rel < 4:  # block straddles the diagonal
                                nc.vector.tensor_mul(
                                    p_sb, p_sb, mask_sb[:, kt_rel, :]
                                )
                            if kt % 2 == 0:
                                p_hold = p_sb
                            else:
                                padd = work.tile(
                                    [128, 512], bf16, tag="padd", name="padd",
                                    bufs=3,
                                )
                                nc.vector.tensor_add(padd, p_hold, p_sb)
                                if kt % 4 == 1:
                                    padd_hold = padd
                                else:
                                    pquad = work.tile(
                                        [128, 512], bf16, tag="pquad",
                                        name="pquad", bufs=2,
                                    )
                                    # off the DVE: GpSimd is otherwise idle
                                    nc.gpsimd.tensor_add(pquad, padd_hold, padd)
                                    # ones[128,128] stationary: column sums
                                    # land broadcast across all partitions
                                    nc.tensor.matmul(
                                        ps_sum,
                                        lhsT=ones128,
                                        rhs=pquad,
                                        start=(kt == 3),
                                        stop=(kt == n_kt - 1),
                                    )
                            yield
                            nc.tensor.matmul(
                                ps_y,
                                lhsT=v_all[:, kt, ts(h, 128)],
                                rhs=p_sb,
                                start=(kt == 0),
                                stop=(kt == n_kt - 1),
                            )
                        # 1/denom = exp(-ln(denom)); Ln/Exp share a table set
                        # (DVE reciprocal measures ~3.4us — far too slow)
                        lnt = work.tile(
                            [128, 512], f32, tag="lnt", name="lnt", bufs=2
                        )
                        nc.scalar.activation(lnt, ps_sum, AF.Ln)
                        rb = work.tile(
                            [128, 512], bf16, tag="rb", name="rb", bufs=2
                        )
                        nc.scalar.activation(rb, lnt, AF.Exp, scale=-1.0)
                        nc.vector.tensor_mul(yT[:, h, ts(qc, 512)], ps_y, rb)
                        yield
                    nc.sync.dma_start(out=yb_r[:, h, :], in_=yT[:, h, :])
                    if h in ag_after:
                        emit_ag(ag_after[h])
                        if hooks["on_ag"] is not None:
                            hooks["on_ag"](ag_after[h])

                def chain(*gens):
                    for g in gens:
                        yield from g

                def drive(main, filler, per_step, lead=0):
                    """Advance `filler` per_step[i % len] times after each
                    main step; then drain both."""
                    for _ in range(lead):
                        if next(filler, None) is None:
                            break
                    i = 0
                    for _ in main:
                        for _ in range(per_step[i % len(per_step)]):
                            if next(filler, None) is None:
                                break
                        i += 1
                    for _ in filler:
                        pass

                # ========== phase A: chunks 0, 2, 4 (heads 0-3 + all V lo) ==
                for _ in chain(qk_steps(0), qk_steps(2), v_steps(4)):
                    pass

                # phase B loads on both queues, behind phase-A traffic
                load_chunk(1, nc.sync, granules=8)
                load_chunk(3, nc.scalar, granules=4)
                nc.sync.dma_start(out=wp, in_=wproj_r)
                load_chunk(5, nc.scalar, granules=2)

                # ========== phase B: chunks 1, 3, 5 ⊗ attention h0-3 ========
                drive(
                    chain(qk_steps(1), qk_steps(3), v_steps(5)),
                    chain(*[attn_steps(h) for h in range(4)]),
                    TUNE["attn_per_qkv"],
                )

            # ========== phase C: c_proj waves ⊗ attention h4-7 ==========
            # ygs slot j = global feature block j: slots 8g'+h come from AG
            # group of head h, rank g'.
            with (
                tc.tile_pool(name="stash_pool", bufs=1) as stash_pool,
                tc.tile_pool(name="ygs_pool", bufs=1) as ygs_pool,
            ):
                stash = stash_pool.tile([128, 16, 512], f32, name="stash")
                ygs = ygs_pool.tile([128, 16, T], bf16, name="ygs")

                def load_group(w):
                    hs = AG_HEADS[w]
                    ygth_r = ygth_q[w].rearrange("(j p) t -> p j t", p=128)
                    for r in range(2):
                        # split across the two HWDGE queues
                        eng = nc.scalar if r == 0 else nc.sync
                        for i, h in enumerate(hs):
                            eng.dma_start(
                                out=ygs[:, 8 * r + h, :],
                                in_=ygth_r[:, r * len(hs) + i, :],
                            )

                load_group(0)
                load_group(1)
                hooks["on_ag"] = load_group

                def proj_steps(heads, first, last, half=False):
                    """One c_proj tile per yield (mid-chain yield too when
                    `half`): chain over local+remote copies of `heads`, then
                    stash-accumulate (or final-merge + output DMA on `last`)."""
                    jts = [8 * r + h for h in heads for r in range(2)]
                    mid = (len(jts) + 1) // 2
                    for tt in range(8):
                        for cc in range(2):
                            st = tt * 2 + cc
                            ps = psA.tile(
                                [128, 512], f32, tag="ps", name="ps_proj"
                            )
                            for i, jt in enumerate(jts):
                                if half and i == mid:
                                    yield
                                nc.tensor.matmul(
                                    ps,
                                    lhsT=ygs[:, jt, ts(tt, 128)],
                                    rhs=wp[:, jt, ts(cc, 512)],
                                    start=(i == 0),
                                    stop=(i == len(jts) - 1),
                                )
                            if first:
                                nc.vector.tensor_add(
                                    stash[:, st, :], ps, bp_bc[:, ts(cc, 512)]
                                )
                            elif not last:
                                nc.vector.tensor_add(
                                    stash[:, st, :], ps, stash[:, st, :]
                                )
                            else:
                                pb = work.tile(
                                    [128, 512], bf16, tag="pb", name="pb",
                                    bufs=4,
                                )
                                nc.vector.tensor_add(pb, ps, stash[:, st, :])
                                nc.sync.dma_start(
                                    out=out.ap()[ts(tt, 128), ts(cc, 512)],
                                    in_=pb,
                                )
                            yield

                # W0a {h0,1} starts on the early AG0; W0b {h2,3} once AG1
                # lands; both ⊗ attn h4-6 at 2-matmul granularity.
                # W1 {h4,5} ⊗ attn h7; W2 {h6} covers AG(h7); W3 {h7} tail.
                drive(
                    chain(
                        proj_steps((0, 1), first=True, last=False, half=True),
                        proj_steps((2, 3), first=False, last=False, half=True),
                    ),
                    chain(*[attn_steps(h) for h in (4, 5, 6)]),
                    (1, 1, 0),
                    lead=4,
                )
                drive(
                    proj_steps((4, 5), first=False, last=False),
                    attn_steps(7),
                    (1, 1, 1, 1, 1, 1, 1, 0),
                )
                for _ in proj_steps((6,), first=False, last=False):
                    pass
                for _ in proj_steps((7,), first=False, last=True):
                    pass

    # Pin every activation to the one table set holding Exp+Identity
    # (natural_log_exp_and_others) so the set-picker never inserts
    # ACT_TABLE_LOADs mid-kernel.
    import concourse.bacc as bacc_mod

    orig_tables = bacc_mod.get_activation_tables

    def _pinned_tables(arch):
        tabs = orig_tables(arch)
        return {
            name: (funcs if name == "natural_log_exp_and_others" else set())
            for name, funcs in tabs.items()
        }

    bacc_mod.get_activation_tables = _pinned_tables
    try:
        nc.finalize()
    finally:
        bacc_mod.get_activation_tables = orig_tables
    return nc


def _host_inputs(x, w_attn, b_attn, w_proj, b_proj):
    """Build the 8 per-core input maps."""
    x = np.asarray(x, np.float32)
    w_attn = np.asarray(w_attn, np.float32)
    b_attn = np.asarray(b_attn, np.float32)
    w_proj = np.asarray(w_proj, np.float32)
    b_proj = np.asarray(b_proj, np.float32)

    # rope tables, transposed [d, t], full height:
    # dest = raw * cos2 + t2 with sinB = [s; -s] read at the source window:
    #   t2[0:64] = raw[64:128] * sinB[64:128] = raw_hi * (-s)
    #   t2[64:128] = raw[0:64] * sinB[0:64]   = raw_lo * s
    inv_freq = 1.0 / (ROPE_BASE ** (np.arange(0, D, 2, dtype=np.float32) / D))
    freqs = np.arange(T, dtype=np.float32)[:, None] * inv_freq[None, :]  # [T, 64]
    c_ = np.ascontiguousarray(np.cos(freqs).T)  # [64, T]
    s_ = np.ascontiguousarray(np.sin(freqs).T)
    cosT = np.concatenate([c_, c_], axis=0).astype(BF16)  # [128, T]
    sinB = np.concatenate([s_, -s_], axis=0).astype(BF16)

    # causal mask blocks, transposed [k, q]: block kt_rel r, q chunk of 512
    k_idx = np.arange(128)
    q_idx = np.arange(512)
    maskT = np.zeros((128, 4, 512), np.float32)
    for r in range(4):
        maskT[:, r, :] = ((r * 128 + k_idx)[:, None] <= q_idx[None, :]).astype(
            np.float32
        )
    maskT = maskT.astype(BF16)

    in_maps = []
    for c in range(N_CORES):
        b, g = divmod(c, 2)
        cs = slice(g * 1024, (g + 1) * 1024)
        wq = w_attn[:, 0:C][:, cs]
        wk = w_attn[:, C : 2 * C][:, cs]
        wv = w_attn[:, 2 * C : 3 * C][:, cs]
        bq = b_attn[0:C][cs]
        bk = b_attn[C : 2 * C][cs]
        bvv = b_attn[2 * C : 3 * C][cs]
        in_maps.append(
            {
                "xT": np.ascontiguousarray(x[b].T).astype(BF16),
                "wqkv": np.concatenate([wq, wk, wv], axis=1).astype(BF16),
                "bqk": np.ascontiguousarray(
                    np.concatenate([bq, bk]).reshape(16, 128).T
                ).astype(np.float32),
                "bv": bvv.reshape(1, 1024).astype(np.float32),
                "cosT": cosT,
                "sinT": sinB,
                "maskT": maskT,
                "wproj": w_proj[:, cs].astype(BF16),
                "bproj": b_proj[cs].reshape(1, 1024).astype(np.float32),
            }
        )
    return in_maps


def kernel(x, w_attn, b_attn, w_proj, b_proj, _trace=False):
    from concourse.bass_utils import run_bass_kernel_spmd

    if "nc" not in _PROGRAM_CACHE:
        _PROGRAM_CACHE["nc"] = _build_program()
    nc = _PROGRAM_CACHE["nc"]

    in_maps = _host_inputs(x, w_attn, b_attn, w_proj, b_proj)
    res = run_bass_kernel_spmd(
        nc, in_maps, core_ids=list(range(N_CORES)), trace=_trace
    )
    _PROGRAM_CACHE["last_results"] = res

    out = np.zeros((B, T, C), np.float32)
    for c in range(N_CORES):
        b, g = divmod(c, 2)
        out[b, :, g * 1024 : (g + 1) * 1024] = np.asarray(
            res.results[c]["out"], dtype=np.float32
        )
    return out
